# revision 2
# baseline (speedup 1.0000x reference)
"""Trainium2 Bass kernel for nn_DecoderRNN (autoregressive LSTM decoder).

Strategy:
  - Pure data parallelism: batch 8192 -> 1024 per core across 8 NeuronCores.
  - Feature-major layout on chip: h^T, c^T are [H=128 partitions, B_local].
    gates^T = W^T blocks (stationary) @ activations (moving), so the
    elementwise LSTM cell update produces h^T directly in the layout the
    next step's matmul needs -- no per-step transposes.
  - The scalar output out_t = W_out @ h_t + b_out feeds the next step's
    input row. That rank-1 contribution is folded into the recurrent
    weights: W~_hh = W_hh + W_ih[:,0:1] @ W_out and
    b~ = b_ih + b_hh + W_ih[:,0] * b_out, which removes the output
    projection from the recurrent critical path entirely. Step 0 uses the
    unfolded W_hh with the observed x value supplied via the z tile.
  - Gate biases ride in the matmul via a constant ones-row appended to the
    z tile (K=65), so PSUM already holds pre-activation gates and the
    ACT engine does pure sigmoid/tanh. ACT is the bottleneck engine
    (5 transcendental passes over [128,1024] per step, ~94% busy), so the
    whole schedule is built to keep its op count minimal and its stream
    bubble-free.
  - Gates are ordered [f, i, o, g]: the three sigmoid gates share ONE
    3-bank PSUM tile per batch half and evict in a single [128,1536] ACT
    op; tanh(g) is a separate 1-bank tile. tanh(c) is one [128,512] op
    per half. 6 ACT ops/step total (vs 10 in the earlier version), saving
    ~870ns/step of fixed per-op overhead (access latency + seq decode).
  - ACT stream order per step: sFIO0, G0, sFIO1, ct0, G1, ct1 -- each
    op's inputs are produced >1 ACT-op-length before its slot, so the
    ACT engine never stalls in steady state.
  - PSUM budget (8 banks): psfio 2 bufs x 3 banks + psg 1 buf x 1 bank +
    po32 accumulator 1 bank = 8.
  - Matmuls run in float32r (single-pass fp32, ~tf32 precision). Gate
    evictions write bf16 (halves DVE cost of i*g via the 2x packed mode);
    c and h stay fp32 for accuracy.
  - The f*c multiply runs on the otherwise-idle GPSIMD engine; i*g, the
    c add and the h mul stay on the (faster) DVE.
  - z-side matmuls for step t+1 are pre-issued during step t (they only
    need the prefetched z tile), so after h_t lands only the h-side
    matmuls gate the next evict.
  - Out rows: step t / half b lands on PSUM partition 32*b + t%32 via
    shifted W_out column blocks, so 64 rows accumulate in one PSUM bank
    and evict once per 32 steps (instead of per-step row copies).
"""

import os
import sys

for _p in ("/opt/trn_rl_repo", "/root/.axon_site/_ro/trn_rl_repo"):
    if os.path.isdir(_p) and _p not in sys.path:
        sys.path.insert(0, _p)

from contextlib import ExitStack

import numpy as np

import concourse.bass as bass  # noqa: F401  (registers types)
import concourse.mybir as mybir
import concourse.tile as tile
from concourse import bacc
from concourse.bass_utils import run_bass_kernel_spmd

NCORES = 8
B, T, F, H, P = 8192, 128, 63, 128, 64
BL = B // NCORES      # 1024 rows per core
I = 2 + F             # 64 LSTM input features + 1 ones-row for bias
G4 = 4 * H            # 512 gate rows
NH = 2                # batch halves (moving-dim chunks of 512)
NW = BL // NH         # 512

_f32 = mybir.dt.float32
_f32r = mybir.dt.float32r
_bf16 = mybir.dt.bfloat16

_CACHE: dict = {}


def _build():
    nc = bacc.Bacc("TRN2", target_bir_lowering=False, debug=False)
    AF = mybir.ActivationFunctionType

    zt_d = nc.dram_tensor("zt", [P, I, BL], _f32r, kind="ExternalInput")
    h0_d = nc.dram_tensor("h0t", [H, BL], _f32r, kind="ExternalInput")
    c0_d = nc.dram_tensor("c0t", [H, BL], _f32, kind="ExternalInput")
    # weight layouts: columns are gate rows permuted to [f, i, o, g]
    wz0_d = nc.dram_tensor("wz0t", [I, G4], _f32r, kind="ExternalInput")
    wzf_d = nc.dram_tensor("wzft", [I, G4], _f32r, kind="ExternalInput")
    wh0_d = nc.dram_tensor("whh0t", [H, G4], _f32r, kind="ExternalInput")
    whf_d = nc.dram_tensor("whhft", [H, G4], _f32r, kind="ExternalInput")
    # out-projection weights: one [128,128] buffer with W_out at column 63;
    # slicing 64 columns starting at 63-(32*half + t%32) yields a block with
    # W_out at column 32*half + t%32, so (step t, half b) lands on PSUM
    # partition 32*b + t%32 and 64 rows accumulate in ONE bank, evicted as
    # a single [64, 512] copy per 32 steps.
    wo_d = nc.dram_tensor("woutt", [H, H], _f32r, kind="ExternalInput")
    out_d = nc.dram_tensor("out", [P, BL], _f32, kind="ExternalOutput")

    with ExitStack() as ctx:
        tc = ctx.enter_context(tile.TileContext(nc))
        const = ctx.enter_context(tc.tile_pool(name="const", bufs=1))
        zp = ctx.enter_context(tc.tile_pool(name="z", bufs=4))
        hp = ctx.enter_context(tc.tile_pool(name="h", bufs=2))
        cp = ctx.enter_context(tc.tile_pool(name="c", bufs=2))
        gp = ctx.enter_context(tc.tile_pool(name="g", bufs=3))
        tp = ctx.enter_context(tc.tile_pool(name="t", bufs=3))
        op = ctx.enter_context(tc.tile_pool(name="osb", bufs=3))
        # PSUM budget (8 banks): 2x 3-bank (f,i,o) tiles + 1x 1-bank g tile
        # + 1x 1-bank 64-row out-projection accumulator.
        psfio = ctx.enter_context(tc.tile_pool(name="psfio", bufs=2, space="PSUM"))
        psg = ctx.enter_context(tc.tile_pool(name="psg", bufs=1, space="PSUM"))
        pspo = ctx.enter_context(tc.tile_pool(name="pspo", bufs=1, space="PSUM"))

        # step-0-critical tensors first so the pipeline fills ASAP; z0/h0
        # arrive as half-width transfers so half-0 matmuls start sooner
        wz0 = const.tile([I, G4], _f32r, tag="wz0")
        nc.sync.dma_start(wz0[:], wz0_d[:])
        zt0 = zp.tile([I, BL], _f32r, tag="z", name="z0")
        nc.sync.dma_start(zt0[:, 0:NW], zt_d[0, :, 0:NW])
        nc.sync.dma_start(zt0[:, NW:BL], zt_d[0, :, NW:BL])
        h_prev = hp.tile([H, BL], _f32r, tag="h")
        nc.sync.dma_start(h_prev[:, 0:NW], h0_d[:, 0:NW])
        nc.sync.dma_start(h_prev[:, NW:BL], h0_d[:, NW:BL])
        wh0 = const.tile([H, G4], _f32r, tag="wh0")
        nc.sync.dma_start(wh0[:], wh0_d[:])
        c_prev = cp.tile([H, BL], _f32, tag="c")
        nc.sync.dma_start(c_prev[:], c0_d[:])
        wzf = const.tile([I, G4], _f32r, tag="wzf")
        nc.sync.dma_start(wzf[:], wzf_d[:])
        whf = const.tile([H, G4], _f32r, tag="whf")
        nc.sync.dma_start(whf[:], whf_d[:])
        wo = const.tile([H, H], _f32r, tag="wo")
        nc.sync.dma_start(wo[:], wo_d[:])

        def z_mms(t, zt, ps):
            """z-side (and bias) matmul contributions for step t; these only
            need the prefetched z tile, so they are emitted during step t-1
            and run while the PE would otherwise wait for h_{t}."""
            wz = wz0 if t == 0 else wzf
            # step 0 runs while the PE clock ramps: half-size mms shorten the
            # slow first instructions. PSUM start=True is bank-granular, so
            # only the FIRST piece per bank carries start=True.
            nq = 2 if t == 0 else 1
            qw = NW // nq
            for half in range(NH):
                psFIO = psfio.tile([H, 3 * NW], _f32, tag="fio",
                                   name=f"psFIO{t}_{half}")
                psG = psg.tile([H, NW], _f32, tag="g", name=f"psG{t}_{half}")
                ps[(t, half)] = (psFIO, psG)
                for j in range(3):          # f, i, o blocks
                    for q in range(nq):
                        js = slice(j * NW + q * qw, j * NW + (q + 1) * qw)
                        qs = slice(half * NW + q * qw, half * NW + (q + 1) * qw)
                        nc.tensor.matmul(psFIO[:, js], wz[:, j * H : (j + 1) * H],
                                         zt[:, qs], start=(q == 0), stop=False)
                for q in range(nq):
                    qs = slice(half * NW + q * qw, half * NW + (q + 1) * qw)
                    qj = slice(q * qw, (q + 1) * qw)
                    nc.tensor.matmul(psG[:, qj], wz[:, 3 * H : 4 * H], zt[:, qs],
                                     start=(q == 0), stop=False)

        ps: dict = {}
        z_mms(0, zt0, ps)

        po32: dict = {}  # half -> PSUM tile accumulating 32 out rows

        # out-row groups (start, len): the last group holds only step 63 so
        # its evict+DMA are tiny; the big 31-row group drains during step 63
        _PO_GROUPS = {}
        for _g0, _glen in ((0, 32), (32, 31), (63, 1)):
            for _t in range(_g0, _g0 + _glen):
                _PO_GROUPS[_t] = (_g0, _glen)

        def emit_po(tp_, h_tile):
            """Out row for (step tp_, half b) lands on PSUM partition
            32*b + (tp_-group_start) via shifted W_out column blocks; a
            group's rows accumulate in ONE bank, one evict per group."""
            g0, glen = _PO_GROUPS[tp_]
            j = tp_ - g0
            if j == 0:
                po32[0] = pspo.tile([64, NW], _f32, tag="po32",
                                    name=f"po32_{tp_}")
            for half in range(NH):
                cs = slice(half * NW, (half + 1) * NW)
                blk = 63 - (half * 32 + j)
                nc.tensor.matmul(po32[0][:], wo[:, blk : blk + 64],
                                 h_tile[:, cs],
                                 start=(j == 0 and half == 0),
                                 stop=(j == glen - 1 and half == NH - 1))
            if j == glen - 1:
                orow32 = op.tile([64, NW], _f32, tag="orow", name=f"orow{tp_}")
                nc.vector.tensor_copy(orow32[:], po32[0][:])
                if glen == 1:
                    # single-row group: one strided-source DMA (both halves)
                    nc.sync.dma_start(out_d[g0 : g0 + 1, :],
                                      orow32[0:64:32, :])
                else:
                    for half in range(NH):
                        cs = slice(half * NW, (half + 1) * NW)
                        nc.sync.dma_start(out_d[g0 : g0 + glen, cs],
                                          orow32[32 * half : 32 * half + glen, :])

        def h_mms(t, half, psFIO, psG):
            wh = wh0 if t == 0 else whf
            cs = slice(half * NW, (half + 1) * NW)
            for j in range(3):          # f, i, o blocks
                js = slice(j * NW, (j + 1) * NW)
                nc.tensor.matmul(psFIO[:, js], wh[:, j * H : (j + 1) * H],
                                 h_prev[:, cs], start=False, stop=True)
            nc.tensor.matmul(psG[:], wh[:, 3 * H : 4 * H], h_prev[:, cs],
                             start=False, stop=True)

        prev = None  # (t, h_tile) pending out-projection
        for t in range(P):
            h_new = hp.tile([H, BL], _f32r, tag="h", name=f"h{t}")
            c_new = cp.tile([H, BL], _f32, tag="c", name=f"c{t}")
            gFIO = [None, None]
            gG = [None, None]

            def evict_fio(half):
                psFIO, _ = ps[(t, half)]
                gFIO[half] = gp.tile([H, 3 * NW], _bf16, tag="gFIO",
                                     name=f"gFIO{t}_{half}")
                nc.scalar.activation(gFIO[half][:], psFIO[:], AF.Sigmoid)

            def evict_g(half):
                _, psG = ps[(t, half)]
                gG[half] = gp.tile([H, NW], _bf16, tag="gG",
                                   name=f"gG{t}_{half}")
                nc.scalar.activation(gG[half][:], psG[:], AF.Tanh)

            def cell_pre(half):
                """f*c on GPSIMD, i*g and the c add on DVE."""
                cs = slice(half * NW, (half + 1) * NW)
                f_s = gFIO[half][:, 0:NW]
                i_s = gFIO[half][:, NW : 2 * NW]
                t1 = tp.tile([H, NW], _f32, tag="t1", name=f"t1_{t}_{half}")
                nc.gpsimd.tensor_mul(t1[:], f_s, c_prev[:, cs])
                t2 = tp.tile([H, NW], _bf16, tag="t2", name=f"t2_{t}_{half}")
                nc.vector.tensor_mul(t2[:], i_s, gG[half][:])
                nc.vector.tensor_add(c_new[:, cs], t1[:], t2[:])

            def cell_post(half):
                """tanh(c) on ACT, then h = o * tanh(c) on DVE."""
                cs = slice(half * NW, (half + 1) * NW)
                o_s = gFIO[half][:, 2 * NW : 3 * NW]
                ct = tp.tile([H, NW], _f32, tag="ct", name=f"ct{t}_{half}")
                nc.scalar.activation(ct[:], c_new[:, cs], AF.Tanh)
                nc.vector.tensor_mul(h_new[:, cs], o_s, ct[:])

            # ACT stream: sFIO0, G0, sFIO1, ct0, G1, ct1 -- every op's
            # inputs land >1 op-length ahead of its slot.
            h_mms(t, 0, *ps[(t, 0)])
            evict_fio(0)
            evict_g(0)
            cell_pre(0)
            h_mms(t, 1, *ps[(t, 1)])
            evict_fio(1)
            cell_post(0)
            evict_g(1)
            cell_pre(1)
            cell_post(1)
            ps.pop((t, 0))
            ps.pop((t, 1))
            # --- prefetch + pre-issue next step's z work on DMA/PE ---
            if t + 1 < P:
                zt = zp.tile([I, BL], _f32r, tag="z", name=f"z{t + 1}")
                nc.sync.dma_start(zt[:], zt_d[t + 1, :, :])
                z_mms(t + 1, zt, ps)
            # out-projection mms last on PE: a po32 bank-release wait at a
            # 32-step group boundary then can't block critical mms behind it
            if prev is not None:
                emit_po(prev[0], prev[1])
            prev = (t, h_new)
            h_prev, c_prev = h_new, c_new
        # final step's out-projection (closes the second 32-group)
        emit_po(prev[0], prev[1])

    nc.compile()
    return nc


def _get_nc():
    if "nc" not in _CACHE:
        _CACHE["nc"] = _build()
    return _CACHE["nc"]


# gate-row permutation: PyTorch order [i,f,g,o] -> kernel order [f,i,o,g]
_PERM = np.concatenate(
    [np.arange(H, 2 * H), np.arange(0, H), np.arange(3 * H, 4 * H),
     np.arange(2 * H, 3 * H)]
)


def _prep_in_maps(x, z, h0, c0, W_ih, W_hh, b_ih, b_hh, W_out, b_out):
    f = np.float32
    Wihp = W_ih[_PERM]                                   # (512, 64)
    Whhp = W_hh[_PERM]                                   # (512, 128)
    Whfp = Whhp + Wihp[:, 0:1] @ W_out                   # fold out-projection
    b0 = (b_ih + b_hh)[_PERM].astype(f)
    bf = (b0 + Wihp[:, 0] * b_out[0]).astype(f)

    # z-side weights with a trailing bias row (matches the ones-row in zt)
    wz0t = np.concatenate([Wihp.T, b0[None, :]], axis=0).astype(f)   # (65, 512)
    wzft = np.concatenate([Wihp.T, bf[None, :]], axis=0).astype(f)   # (65, 512)
    whh0t = np.ascontiguousarray(Whhp.T, dtype=f)                    # (128, 512)
    whhft = np.ascontiguousarray(Whfp.T, dtype=f)                    # (128, 512)
    # W_out at column 63 of a zeros buffer; emit_po slices 64 columns at a
    # shifted offset so W_out lands on the right PSUM partition
    woutt = np.zeros((H, H), dtype=f)
    woutt[:, 63] = W_out[0]

    in_maps = []
    for m in range(NCORES):
        sl = slice(m * BL, (m + 1) * BL)
        z_aug = np.empty((P, I, BL), dtype=f)
        z_aug[:, 0, :] = 0.0
        z_aug[0, 0, :] = x[sl, -1, 0]
        z_aug[:, 1:-1, :] = np.transpose(z[sl, T - P :, :], (1, 2, 0))
        z_aug[:, -1, :] = 1.0
        in_maps.append(
            {
                "zt": np.ascontiguousarray(z_aug),
                "h0t": np.ascontiguousarray(h0[0, sl, :].T, dtype=f),
                "c0t": np.ascontiguousarray(c0[0, sl, :].T, dtype=f),
                "wz0t": wz0t,
                "wzft": wzft,
                "whh0t": whh0t,
                "whhft": whhft,
                "woutt": woutt,
            }
        )
    return in_maps


def run_on_cores(inputs: dict, **spmd_kwargs):
    """Build + run; returns (full_output, BassKernelResults)."""
    inputs = {k: np.asarray(v, dtype=np.float32) for k, v in inputs.items()}
    nc = _get_nc()
    in_maps = _prep_in_maps(**inputs)
    res = run_bass_kernel_spmd(nc, in_maps, core_ids=list(range(NCORES)), **spmd_kwargs)
    outs = np.concatenate(
        [r["out"].T for r in res.results], axis=0
    )  # (8192, 64)
    outs = outs + np.float32(inputs["b_out"][0])
    return outs[:, :, None].astype(np.float32), res


def kernel(**inputs) -> np.ndarray:
    out, _ = run_on_cores(inputs)
    return out


# revision 5
# speedup vs baseline: 1.2751x; 1.2751x over previous
"""Trainium2 Bass kernel for nn_DecoderRNN (autoregressive LSTM decoder).

Strategy:
  - Pure data parallelism: batch 8192 -> 1024 per core across 8 NeuronCores.
  - Feature-major layout on chip: h^T, c^T are [H=128 partitions, B_local].
    gates^T = W^T blocks (stationary) @ activations (moving), so the
    elementwise LSTM cell update produces h^T directly in the layout the
    next step's matmul needs -- no per-step transposes.
  - Output projection folded into the recurrent weights (W~_hh = W_hh +
    W_ih[:,0:1] @ W_out); biases ride in the matmul via a ones-row in the
    z tile. Step 0 uses unfolded weights with x supplied via the z tile.
  - ACT (scalar engine) is the bottleneck: 5 transcendental passes over
    [128,1024] per step = 4.27us/step of column time at 1.2GHz, plus
    ~185ns fixed cost per activation instruction. The schedule targets
    exactly 8 ACT ops/step (sFI, G, sO, ct per half) in the stream order
      sFI0 G0 sO0 sFI1 G1 ct0 sO1 ct1
    which keeps every op's inputs ready >= its slot start and closes the
    per-half recurrence cycle (evict -> cell -> tanh(c) -> h -> h-side
    matmul -> next evict) in exactly the 6.0us/step ACT busy time.
  - Gate order [f, i, g, o]: f,i share a 2-bank PSUM tile (one sigmoid
    evict); g and o are 1-bank tiles. tanh(c) is one [128,512] op per
    half (not quartered: fewer ACT ops wins over finer pipelining).
  - Gate evictions write bf16: i*g runs in the DVE 2x packed mode, and
    h = o*tanh(c) is all-bf16 (h feeds the matmuls as bf16 moving data,
    same 1 cycle/row as f32r). c and f*c stay fp32 for accuracy.
  - All three cell ops (f*c, i*g, add) run on DVE -- GPSIMD's 0.42x
    multiply efficiency makes it too slow for the critical path; it only
    gets the out-row PSUM->SBUF copies.
  - z-side matmuls for step t+1 are pre-issued during step t; h-side
    matmuls are emitted f,i,g,o so the sFI evict unblocks after two mms.
  - Out rows: step t / half b lands on PSUM partition 32*b + t%32 via
    shifted W_out column blocks, so 64 rows accumulate in one PSUM bank
    and evict once per 32 steps.
  - PSUM budget (8 banks): FI 2bufs x 2 + G 2bufs x 1 + O 1buf x 1 +
    po32 1 = 8.
"""

import os
import sys

for _p in ("/opt/trn_rl_repo", "/root/.axon_site/_ro/trn_rl_repo"):
    if os.path.isdir(_p) and _p not in sys.path:
        sys.path.insert(0, _p)

from contextlib import ExitStack

import numpy as np

import concourse.bass as bass  # noqa: F401  (registers types)
import concourse.mybir as mybir
import concourse.tile as tile
from concourse import bacc
from concourse.bass_utils import run_bass_kernel_spmd

NCORES = 8
B, T, F, H, P = 8192, 128, 63, 128, 64
BL = B // NCORES      # 1024 rows per core
I = 2 + F             # 64 LSTM input features + 1 ones-row for bias
G4 = 4 * H            # 512 gate rows
NH = 2                # batch halves (moving-dim chunks of 512)
NW = BL // NH         # 512

_f32 = mybir.dt.float32
_f32r = mybir.dt.float32r
_bf16 = mybir.dt.bfloat16

_CACHE: dict = {}


def _build():
    nc = bacc.Bacc("TRN2", target_bir_lowering=False, debug=False)
    AF = mybir.ActivationFunctionType

    zt_d = nc.dram_tensor("zt", [P, I, BL], _f32r, kind="ExternalInput")
    h0_d = nc.dram_tensor("h0t", [H, BL], _bf16, kind="ExternalInput")
    c0_d = nc.dram_tensor("c0t", [H, BL], _f32, kind="ExternalInput")
    # weight layouts: columns are gate rows permuted to [f, i, g, o]
    wz0_d = nc.dram_tensor("wz0t", [I, G4], _f32r, kind="ExternalInput")
    wzf_d = nc.dram_tensor("wzft", [I, G4], _f32r, kind="ExternalInput")
    wh0_d = nc.dram_tensor("whh0t", [H, G4], _bf16, kind="ExternalInput")
    whf_d = nc.dram_tensor("whhft", [H, G4], _bf16, kind="ExternalInput")
    wo_d = nc.dram_tensor("woutt", [H, H], _bf16, kind="ExternalInput")
    out_d = nc.dram_tensor("out", [P, BL], _f32, kind="ExternalOutput")

    with ExitStack() as ctx:
        tc = ctx.enter_context(tile.TileContext(nc))
        const = ctx.enter_context(tc.tile_pool(name="const", bufs=1))
        zp = ctx.enter_context(tc.tile_pool(name="z", bufs=4))
        hp = ctx.enter_context(tc.tile_pool(name="h", bufs=2))
        cp = ctx.enter_context(tc.tile_pool(name="c", bufs=2))
        gp = ctx.enter_context(tc.tile_pool(name="g", bufs=3))
        tp = ctx.enter_context(tc.tile_pool(name="t", bufs=3))
        op = ctx.enter_context(tc.tile_pool(name="osb", bufs=3))
        # PSUM budget (8 banks): FI 2x2 + G 2x1 + O 1x1 + po32 1 = 8
        psfi = ctx.enter_context(tc.tile_pool(name="psfi", bufs=2, space="PSUM"))
        psg = ctx.enter_context(tc.tile_pool(name="psg", bufs=2, space="PSUM"))
        pso = ctx.enter_context(tc.tile_pool(name="pso", bufs=1, space="PSUM"))
        pspo = ctx.enter_context(tc.tile_pool(name="pspo", bufs=1, space="PSUM"))

        # step-0-critical tensors first so the pipeline fills ASAP
        wz0 = const.tile([I, G4], _f32r, tag="wz0")
        nc.sync.dma_start(wz0[:], wz0_d[:])
        zt0 = zp.tile([I, BL], _f32r, tag="z", name="z0")
        nc.sync.dma_start(zt0[:, 0:NW], zt_d[0, :, 0:NW])
        nc.sync.dma_start(zt0[:, NW:BL], zt_d[0, :, NW:BL])
        h_prev = hp.tile([H, BL], _bf16, tag="h")
        nc.sync.dma_start(h_prev[:, 0:NW], h0_d[:, 0:NW])
        nc.sync.dma_start(h_prev[:, NW:BL], h0_d[:, NW:BL])
        wh0 = const.tile([H, G4], _bf16, tag="wh0")
        nc.sync.dma_start(wh0[:], wh0_d[:])
        c_prev = cp.tile([H, BL], _f32, tag="c")
        nc.sync.dma_start(c_prev[:], c0_d[:])
        wzf = const.tile([I, G4], _f32r, tag="wzf")
        nc.sync.dma_start(wzf[:], wzf_d[:])
        whf = const.tile([H, G4], _bf16, tag="whf")
        nc.sync.dma_start(whf[:], whf_d[:])
        wo = const.tile([H, H], _bf16, tag="wo")
        nc.sync.dma_start(wo[:], wo_d[:])

        def z_mms(t, zt, ps):
            """z-side (and bias) matmul contributions for step t; emitted
            during step t-1, they run while the PE waits for h_t."""
            wz = wz0 if t == 0 else wzf
            nq = 2 if t == 0 else 1
            qw = NW // nq
            for half in range(NH):
                psFI = psfi.tile([H, 2 * NW], _f32, tag="fi",
                                 name=f"psFI{t}_{half}")
                psG = psg.tile([H, NW], _f32, tag="g", name=f"psG{t}_{half}")
                psO = pso.tile([H, NW], _f32, tag="o", name=f"psO{t}_{half}")
                ps[(t, half)] = (psFI, psG, psO)
                for j in range(2):      # f, i blocks
                    for q in range(nq):
                        js = slice(j * NW + q * qw, j * NW + (q + 1) * qw)
                        qs = slice(half * NW + q * qw, half * NW + (q + 1) * qw)
                        nc.tensor.matmul(psFI[:, js], wz[:, j * H : (j + 1) * H],
                                         zt[:, qs], start=(q == 0), stop=False)
                for q in range(nq):
                    qs = slice(half * NW + q * qw, half * NW + (q + 1) * qw)
                    qj = slice(q * qw, (q + 1) * qw)
                    nc.tensor.matmul(psG[:, qj], wz[:, 2 * H : 3 * H], zt[:, qs],
                                     start=(q == 0), stop=False)
                    nc.tensor.matmul(psO[:, qj], wz[:, 3 * H : 4 * H], zt[:, qs],
                                     start=(q == 0), stop=False)

        ps: dict = {}
        z_mms(0, zt0, ps)

        po32: dict = {}

        _PO_GROUPS = {}
        for _g0, _glen in ((0, 32), (32, 31), (63, 1)):
            for _t in range(_g0, _g0 + _glen):
                _PO_GROUPS[_t] = (_g0, _glen)

        def emit_po(tp_, h_tile):
            g0, glen = _PO_GROUPS[tp_]
            j = tp_ - g0
            if j == 0:
                po32[0] = pspo.tile([64, NW], _f32, tag="po32",
                                    name=f"po32_{tp_}")
            for half in range(NH):
                cs = slice(half * NW, (half + 1) * NW)
                blk = 63 - (half * 32 + j)
                nc.tensor.matmul(po32[0][:], wo[:, blk : blk + 64],
                                 h_tile[:, cs],
                                 start=(j == 0 and half == 0),
                                 stop=(j == glen - 1 and half == NH - 1))
            if j == glen - 1:
                orow32 = op.tile([64, NW], _f32, tag="orow", name=f"orow{tp_}")
                nc.vector.tensor_copy(orow32[:], po32[0][:])
                if glen == 1:
                    nc.sync.dma_start(out_d[g0 : g0 + 1, :],
                                      orow32[0:64:32, :])
                else:
                    for half in range(NH):
                        cs = slice(half * NW, (half + 1) * NW)
                        nc.sync.dma_start(out_d[g0 : g0 + glen, cs],
                                          orow32[32 * half : 32 * half + glen, :])

        def h_mms(t, half, psFI, psG, psO):
            """h-side matmuls in f,i,g,o order: sFI unblocks after 2 mms."""
            wh = wh0 if t == 0 else whf
            cs = slice(half * NW, (half + 1) * NW)
            for j in range(2):
                js = slice(j * NW, (j + 1) * NW)
                nc.tensor.matmul(psFI[:, js], wh[:, j * H : (j + 1) * H],
                                 h_prev[:, cs], start=False, stop=True)
            nc.tensor.matmul(psG[:], wh[:, 2 * H : 3 * H], h_prev[:, cs],
                             start=False, stop=True)
            nc.tensor.matmul(psO[:], wh[:, 3 * H : 4 * H], h_prev[:, cs],
                             start=False, stop=True)

        prev = None  # (t, h_tile) pending out-projection
        for t in range(P):
            h_new = hp.tile([H, BL], _bf16, tag="h", name=f"h{t}")
            c_new = cp.tile([H, BL], _f32, tag="c", name=f"c{t}")
            gFI = [None, None]
            gG = [None, None]
            gO = [None, None]
            ct = [None, None]

            def evict_fi(half):
                psFI, _, _ = ps[(t, half)]
                gFI[half] = gp.tile([H, 2 * NW], _bf16, tag="gFI",
                                    name=f"gFI{t}_{half}")
                nc.scalar.activation(gFI[half][:], psFI[:], AF.Sigmoid)

            def evict_g(half):
                _, psG, _ = ps[(t, half)]
                gG[half] = gp.tile([H, NW], _bf16, tag="gG",
                                   name=f"gG{t}_{half}")
                nc.scalar.activation(gG[half][:], psG[:], AF.Tanh)

            def evict_o(half):
                _, _, psO = ps[(t, half)]
                gO[half] = gp.tile([H, NW], _bf16, tag="gO",
                                   name=f"gO{t}_{half}")
                nc.scalar.activation(gO[half][:], psO[:], AF.Sigmoid)

            def cell(half):
                """c = f*c_prev + i*g, all on DVE (t1 fp32, t2 bf16 2x)."""
                cs = slice(half * NW, (half + 1) * NW)
                f_s = gFI[half][:, 0:NW]
                i_s = gFI[half][:, NW : 2 * NW]
                t1 = tp.tile([H, NW], _f32, tag="t1", name=f"t1_{t}_{half}")
                nc.vector.tensor_mul(t1[:], f_s, c_prev[:, cs])
                t2 = tp.tile([H, NW], _bf16, tag="t2", name=f"t2_{t}_{half}")
                nc.vector.tensor_mul(t2[:], i_s, gG[half][:])
                nc.vector.tensor_add(c_new[:, cs], t1[:], t2[:])

            def tanh_c(half):
                cs = slice(half * NW, (half + 1) * NW)
                ct[half] = tp.tile([H, NW], _bf16, tag="ct",
                                   name=f"ct{t}_{half}")
                nc.scalar.activation(ct[half][:], c_new[:, cs], AF.Tanh)

            def h_mul(half):
                cs = slice(half * NW, (half + 1) * NW)
                o_s = gO[half][:]
                nc.vector.tensor_mul(h_new[:, cs], o_s, ct[half][:])

            # ACT priority order: sFI0 G0 sO0 sFI1 G1 ct0 sO1 ct1
            h_mms(t, 0, *ps[(t, 0)])
            evict_fi(0)
            evict_g(0)
            evict_o(0)
            cell(0)
            h_mms(t, 1, *ps[(t, 1)])
            evict_fi(1)
            evict_g(1)
            tanh_c(0)
            h_mul(0)
            evict_o(1)
            cell(1)
            tanh_c(1)
            h_mul(1)
            ps.pop((t, 0))
            ps.pop((t, 1))
            if t + 1 < P:
                zt = zp.tile([I, BL], _f32r, tag="z", name=f"z{t + 1}")
                nc.sync.dma_start(zt[:], zt_d[t + 1, :, :])
                z_mms(t + 1, zt, ps)
            if prev is not None:
                emit_po(prev[0], prev[1])
            prev = (t, h_new)
            h_prev, c_prev = h_new, c_new
        emit_po(prev[0], prev[1])

    nc.compile()
    return nc


def _get_nc():
    if "nc" not in _CACHE:
        _CACHE["nc"] = _build()
    return _CACHE["nc"]


# gate-row permutation: PyTorch order [i,f,g,o] -> kernel order [f,i,g,o]
_PERM = np.concatenate(
    [np.arange(H, 2 * H), np.arange(0, H), np.arange(2 * H, 3 * H),
     np.arange(3 * H, 4 * H)]
)


def _prep_in_maps(x, z, h0, c0, W_ih, W_hh, b_ih, b_hh, W_out, b_out):
    f = np.float32
    Wihp = W_ih[_PERM]                                   # (512, 64)
    Whhp = W_hh[_PERM]                                   # (512, 128)
    Whfp = Whhp + Wihp[:, 0:1] @ W_out                   # fold out-projection
    b0 = (b_ih + b_hh)[_PERM].astype(f)
    bf = (b0 + Wihp[:, 0] * b_out[0]).astype(f)

    wz0t = np.concatenate([Wihp.T, b0[None, :]], axis=0).astype(f)   # (65, 512)
    wzft = np.concatenate([Wihp.T, bf[None, :]], axis=0).astype(f)   # (65, 512)
    whh0t = np.ascontiguousarray(Whhp.T, dtype=f)                    # (128, 512)
    whhft = np.ascontiguousarray(Whfp.T, dtype=f)                    # (128, 512)
    woutt = np.zeros((H, H), dtype=f)
    woutt[:, 63] = W_out[0]

    import ml_dtypes
    in_maps = []
    for m in range(NCORES):
        sl = slice(m * BL, (m + 1) * BL)
        z_aug = np.empty((P, I, BL), dtype=f)
        z_aug[:, 0, :] = 0.0
        z_aug[0, 0, :] = x[sl, -1, 0]
        z_aug[:, 1:-1, :] = np.transpose(z[sl, T - P :, :], (1, 2, 0))
        z_aug[:, -1, :] = 1.0
        in_maps.append(
            {
                "zt": np.ascontiguousarray(z_aug),
                "h0t": np.ascontiguousarray(h0[0, sl, :].T).astype(
                    ml_dtypes.bfloat16),
                "c0t": np.ascontiguousarray(c0[0, sl, :].T, dtype=f),
                "wz0t": wz0t,
                "wzft": wzft,
                "whh0t": whh0t.astype(ml_dtypes.bfloat16),
                "whhft": whhft.astype(ml_dtypes.bfloat16),
                "woutt": woutt.astype(ml_dtypes.bfloat16),
            }
        )
    return in_maps


def run_on_cores(inputs: dict, **spmd_kwargs):
    """Build + run; returns (full_output, BassKernelResults)."""
    inputs = {k: np.asarray(v, dtype=np.float32) for k, v in inputs.items()}
    nc = _get_nc()
    in_maps = _prep_in_maps(**inputs)
    res = run_bass_kernel_spmd(nc, in_maps, core_ids=list(range(NCORES)), **spmd_kwargs)
    outs = np.concatenate(
        [r["out"].T for r in res.results], axis=0
    )  # (8192, 64)
    outs = outs + np.float32(inputs["b_out"][0])
    return outs[:, :, None].astype(np.float32), res


def kernel(**inputs) -> np.ndarray:
    out, _ = run_on_cores(inputs)
    return out


# revision 6
# speedup vs baseline: 1.2866x; 1.0090x over previous
"""Trainium2 Bass kernel for nn_DecoderRNN (autoregressive LSTM decoder).

Strategy:
  - Pure data parallelism: batch 8192 -> 1024 per core across 8 NeuronCores.
  - Feature-major layout on chip: h^T, c^T are [H=128 partitions, B_local].
    gates^T = W^T blocks (stationary) @ activations (moving), so the
    elementwise LSTM cell update produces h^T directly in the layout the
    next step's matmul needs -- no per-step transposes.
  - Output projection folded into the recurrent weights (W~_hh = W_hh +
    W_ih[:,0:1] @ W_out); biases ride in the matmul via a ones-row in the
    z tile. Step 0 uses unfolded weights with x supplied via the z tile.
  - ACT (scalar engine) is the bottleneck: 5 transcendental passes over
    [128,1024] per step = 4.27us/step of column time at 1.2GHz, plus
    ~185ns fixed cost per activation instruction. The schedule targets
    exactly 8 ACT ops/step (sFI, G, sO, ct per half) in the stream order
      sFI0 G0 sO0 sFI1 G1 ct0 sO1 ct1
    which keeps every op's inputs ready >= its slot start and closes the
    per-half recurrence cycle (evict -> cell -> tanh(c) -> h -> h-side
    matmul -> next evict) in exactly the 6.0us/step ACT busy time.
  - Gate order [f, i, g, o]: f,i share a 2-bank PSUM tile (one sigmoid
    evict); g and o are 1-bank tiles. tanh(c) is one [128,512] op per
    half (not quartered: fewer ACT ops wins over finer pipelining).
  - Gate evictions write bf16: i*g runs in the DVE 2x packed mode, and
    h = o*tanh(c) is all-bf16 (h feeds the matmuls as bf16 moving data,
    same 1 cycle/row as f32r). c and f*c stay fp32 for accuracy.
  - All three cell ops (f*c, i*g, add) run on DVE -- GPSIMD's 0.42x
    multiply efficiency makes it too slow for the critical path; it only
    gets the out-row PSUM->SBUF copies.
  - z-side matmuls for step t+1 are pre-issued during step t; h-side
    matmuls are emitted f,i,g,o so the sFI evict unblocks after two mms.
  - Out rows: step t / half b lands on PSUM partition 32*b + t%32 via
    shifted W_out column blocks, so 64 rows accumulate in one PSUM bank
    and evict once per 32 steps.
  - PSUM budget (8 banks): FI 2bufs x 2 + G 2bufs x 1 + O 1buf x 1 +
    po32 1 = 8.
"""

import os
import sys

for _p in ("/opt/trn_rl_repo", "/root/.axon_site/_ro/trn_rl_repo"):
    if os.path.isdir(_p) and _p not in sys.path:
        sys.path.insert(0, _p)

from contextlib import ExitStack

import numpy as np

import concourse.bass as bass  # noqa: F401  (registers types)
import concourse.mybir as mybir
import concourse.tile as tile
from concourse import bacc
from concourse.bass_utils import run_bass_kernel_spmd

NCORES = 8
B, T, F, H, P = 8192, 128, 63, 128, 64
BL = B // NCORES      # 1024 rows per core
I = 2 + F             # 64 LSTM input features + 1 ones-row for bias
G4 = 4 * H            # 512 gate rows
NH = 2                # batch halves (moving-dim chunks of 512)
NW = BL // NH         # 512

_f32 = mybir.dt.float32
_f32r = mybir.dt.float32r
_bf16 = mybir.dt.bfloat16

_CACHE: dict = {}


def _build():
    nc = bacc.Bacc("TRN2", target_bir_lowering=False, debug=False)
    AF = mybir.ActivationFunctionType

    zt_d = nc.dram_tensor("zt", [P, I, BL], _f32r, kind="ExternalInput")
    h0_d = nc.dram_tensor("h0t", [H, BL], _bf16, kind="ExternalInput")
    c0_d = nc.dram_tensor("c0t", [H, BL], _bf16, kind="ExternalInput")
    # weight layouts: columns are gate rows permuted to [f, i, g, o]
    wz0_d = nc.dram_tensor("wz0t", [I, G4], _f32r, kind="ExternalInput")
    wzf_d = nc.dram_tensor("wzft", [I, G4], _f32r, kind="ExternalInput")
    wh0_d = nc.dram_tensor("whh0t", [H, G4], _bf16, kind="ExternalInput")
    whf_d = nc.dram_tensor("whhft", [H, G4], _bf16, kind="ExternalInput")
    wo_d = nc.dram_tensor("woutt", [H, H], _bf16, kind="ExternalInput")
    out_d = nc.dram_tensor("out", [P, BL], _f32, kind="ExternalOutput")

    with ExitStack() as ctx:
        tc = ctx.enter_context(tile.TileContext(nc))
        const = ctx.enter_context(tc.tile_pool(name="const", bufs=1))
        zp = ctx.enter_context(tc.tile_pool(name="z", bufs=4))
        hp = ctx.enter_context(tc.tile_pool(name="h", bufs=2))
        cp = ctx.enter_context(tc.tile_pool(name="c", bufs=2))
        gp = ctx.enter_context(tc.tile_pool(name="g", bufs=3))
        tp = ctx.enter_context(tc.tile_pool(name="t", bufs=3))
        op = ctx.enter_context(tc.tile_pool(name="osb", bufs=3))
        # PSUM budget (8 banks): FI 2x2 + G 2x1 + O 1x1 + po32 1 = 8
        psfi = ctx.enter_context(tc.tile_pool(name="psfi", bufs=2, space="PSUM"))
        psg = ctx.enter_context(tc.tile_pool(name="psg", bufs=2, space="PSUM"))
        pso = ctx.enter_context(tc.tile_pool(name="pso", bufs=1, space="PSUM"))
        pspo = ctx.enter_context(tc.tile_pool(name="pspo", bufs=1, space="PSUM"))

        # step-0-critical tensors first so the pipeline fills ASAP
        wz0 = const.tile([I, G4], _f32r, tag="wz0")
        nc.sync.dma_start(wz0[:], wz0_d[:])
        zt0 = zp.tile([I, BL], _f32r, tag="z", name="z0")
        nc.sync.dma_start(zt0[:, 0:NW], zt_d[0, :, 0:NW])
        nc.sync.dma_start(zt0[:, NW:BL], zt_d[0, :, NW:BL])
        h_prev = hp.tile([H, BL], _bf16, tag="h")
        nc.sync.dma_start(h_prev[:, 0:NW], h0_d[:, 0:NW])
        nc.sync.dma_start(h_prev[:, NW:BL], h0_d[:, NW:BL])
        wh0 = const.tile([H, G4], _bf16, tag="wh0")
        nc.sync.dma_start(wh0[:], wh0_d[:])
        c_prev = cp.tile([H, BL], _bf16, tag="c")
        nc.sync.dma_start(c_prev[:], c0_d[:])
        wzf = const.tile([I, G4], _f32r, tag="wzf")
        nc.sync.dma_start(wzf[:], wzf_d[:])
        whf = const.tile([H, G4], _bf16, tag="whf")
        nc.sync.dma_start(whf[:], whf_d[:])
        wo = const.tile([H, H], _bf16, tag="wo")
        nc.sync.dma_start(wo[:], wo_d[:])

        def z_mms(t, zt, ps):
            """z-side (and bias) matmul contributions for step t; emitted
            during step t-1, they run while the PE waits for h_t."""
            wz = wz0 if t == 0 else wzf
            nq = 2 if t == 0 else 1
            qw = NW // nq
            for half in range(NH):
                psFI = psfi.tile([H, 2 * NW], _f32, tag="fi",
                                 name=f"psFI{t}_{half}")
                psG = psg.tile([H, NW], _f32, tag="g", name=f"psG{t}_{half}")
                psO = pso.tile([H, NW], _f32, tag="o", name=f"psO{t}_{half}")
                ps[(t, half)] = (psFI, psG, psO)
                for j in range(2):      # f, i blocks
                    for q in range(nq):
                        js = slice(j * NW + q * qw, j * NW + (q + 1) * qw)
                        qs = slice(half * NW + q * qw, half * NW + (q + 1) * qw)
                        nc.tensor.matmul(psFI[:, js], wz[:, j * H : (j + 1) * H],
                                         zt[:, qs], start=(q == 0), stop=False)
                for q in range(nq):
                    qs = slice(half * NW + q * qw, half * NW + (q + 1) * qw)
                    qj = slice(q * qw, (q + 1) * qw)
                    nc.tensor.matmul(psG[:, qj], wz[:, 2 * H : 3 * H], zt[:, qs],
                                     start=(q == 0), stop=False)
                    nc.tensor.matmul(psO[:, qj], wz[:, 3 * H : 4 * H], zt[:, qs],
                                     start=(q == 0), stop=False)

        ps: dict = {}
        z_mms(0, zt0, ps)

        po32: dict = {}

        _PO_GROUPS = {}
        for _g0, _glen in ((0, 32), (32, 31), (63, 1)):
            for _t in range(_g0, _g0 + _glen):
                _PO_GROUPS[_t] = (_g0, _glen)

        def emit_po(tp_, h_tile):
            g0, glen = _PO_GROUPS[tp_]
            j = tp_ - g0
            if j == 0:
                po32[0] = pspo.tile([64, NW], _f32, tag="po32",
                                    name=f"po32_{tp_}")
            for half in range(NH):
                cs = slice(half * NW, (half + 1) * NW)
                blk = 63 - (half * 32 + j)
                nc.tensor.matmul(po32[0][:], wo[:, blk : blk + 64],
                                 h_tile[:, cs],
                                 start=(j == 0 and half == 0),
                                 stop=(j == glen - 1 and half == NH - 1))
            if j == glen - 1:
                orow32 = op.tile([64, NW], _f32, tag="orow", name=f"orow{tp_}")
                nc.vector.tensor_copy(orow32[:], po32[0][:])
                if glen == 1:
                    nc.sync.dma_start(out_d[g0 : g0 + 1, :],
                                      orow32[0:64:32, :])
                else:
                    for half in range(NH):
                        cs = slice(half * NW, (half + 1) * NW)
                        nc.sync.dma_start(out_d[g0 : g0 + glen, cs],
                                          orow32[32 * half : 32 * half + glen, :])

        def h_mms(t, half, psFI, psG, psO):
            """h-side matmuls in f,i,g,o order: sFI unblocks after 2 mms."""
            wh = wh0 if t == 0 else whf
            cs = slice(half * NW, (half + 1) * NW)
            for j in range(2):
                js = slice(j * NW, (j + 1) * NW)
                nc.tensor.matmul(psFI[:, js], wh[:, j * H : (j + 1) * H],
                                 h_prev[:, cs], start=False, stop=True)
            nc.tensor.matmul(psG[:], wh[:, 2 * H : 3 * H], h_prev[:, cs],
                             start=False, stop=True)
            nc.tensor.matmul(psO[:], wh[:, 3 * H : 4 * H], h_prev[:, cs],
                             start=False, stop=True)

        prev = None  # (t, h_tile) pending out-projection
        for t in range(P):
            h_new = hp.tile([H, BL], _bf16, tag="h", name=f"h{t}")
            c_new = cp.tile([H, BL], _bf16, tag="c", name=f"c{t}")
            gFI = [None, None]
            gG = [None, None]
            gO = [None, None]
            ct = [None, None]

            def evict_fi(half):
                psFI, _, _ = ps[(t, half)]
                gFI[half] = gp.tile([H, 2 * NW], _bf16, tag="gFI",
                                    name=f"gFI{t}_{half}")
                nc.scalar.activation(gFI[half][:], psFI[:], AF.Sigmoid)

            def evict_g(half):
                _, psG, _ = ps[(t, half)]
                gG[half] = gp.tile([H, NW], _bf16, tag="gG",
                                   name=f"gG{t}_{half}")
                nc.scalar.activation(gG[half][:], psG[:], AF.Tanh)

            def evict_o(half):
                _, _, psO = ps[(t, half)]
                gO[half] = gp.tile([H, NW], _bf16, tag="gO",
                                   name=f"gO{t}_{half}")
                nc.scalar.activation(gO[half][:], psO[:], AF.Sigmoid)

            def cell(half):
                """c = f*c_prev + i*g, all on DVE (t1 fp32, t2 bf16 2x)."""
                cs = slice(half * NW, (half + 1) * NW)
                f_s = gFI[half][:, 0:NW]
                i_s = gFI[half][:, NW : 2 * NW]
                t1 = tp.tile([H, NW], _bf16, tag="t1", name=f"t1_{t}_{half}")
                nc.vector.tensor_mul(t1[:], f_s, c_prev[:, cs])
                t2 = tp.tile([H, NW], _bf16, tag="t2", name=f"t2_{t}_{half}")
                nc.vector.tensor_mul(t2[:], i_s, gG[half][:])
                nc.vector.tensor_add(c_new[:, cs], t1[:], t2[:])

            def tanh_c(half):
                cs = slice(half * NW, (half + 1) * NW)
                ct[half] = tp.tile([H, NW], _bf16, tag="ct",
                                   name=f"ct{t}_{half}")
                nc.scalar.activation(ct[half][:], c_new[:, cs], AF.Tanh)

            def h_mul(half):
                cs = slice(half * NW, (half + 1) * NW)
                o_s = gO[half][:]
                nc.vector.tensor_mul(h_new[:, cs], o_s, ct[half][:])

            # ACT priority order: sFI0 G0 sO0 sFI1 G1 ct0 sO1 ct1
            h_mms(t, 0, *ps[(t, 0)])
            evict_fi(0)
            evict_g(0)
            evict_o(0)
            cell(0)
            h_mms(t, 1, *ps[(t, 1)])
            evict_fi(1)
            evict_g(1)
            tanh_c(0)
            cell(1)
            h_mul(0)
            evict_o(1)
            tanh_c(1)
            h_mul(1)
            ps.pop((t, 0))
            ps.pop((t, 1))
            if t + 1 < P:
                zt = zp.tile([I, BL], _f32r, tag="z", name=f"z{t + 1}")
                nc.sync.dma_start(zt[:], zt_d[t + 1, :, :])
                z_mms(t + 1, zt, ps)
            if prev is not None:
                emit_po(prev[0], prev[1])
            prev = (t, h_new)
            h_prev, c_prev = h_new, c_new
        emit_po(prev[0], prev[1])

    nc.compile()
    return nc


def _get_nc():
    if "nc" not in _CACHE:
        _CACHE["nc"] = _build()
    return _CACHE["nc"]


# gate-row permutation: PyTorch order [i,f,g,o] -> kernel order [f,i,g,o]
_PERM = np.concatenate(
    [np.arange(H, 2 * H), np.arange(0, H), np.arange(2 * H, 3 * H),
     np.arange(3 * H, 4 * H)]
)


def _prep_in_maps(x, z, h0, c0, W_ih, W_hh, b_ih, b_hh, W_out, b_out):
    f = np.float32
    Wihp = W_ih[_PERM]                                   # (512, 64)
    Whhp = W_hh[_PERM]                                   # (512, 128)
    Whfp = Whhp + Wihp[:, 0:1] @ W_out                   # fold out-projection
    b0 = (b_ih + b_hh)[_PERM].astype(f)
    bf = (b0 + Wihp[:, 0] * b_out[0]).astype(f)

    wz0t = np.concatenate([Wihp.T, b0[None, :]], axis=0).astype(f)   # (65, 512)
    wzft = np.concatenate([Wihp.T, bf[None, :]], axis=0).astype(f)   # (65, 512)
    whh0t = np.ascontiguousarray(Whhp.T, dtype=f)                    # (128, 512)
    whhft = np.ascontiguousarray(Whfp.T, dtype=f)                    # (128, 512)
    woutt = np.zeros((H, H), dtype=f)
    woutt[:, 63] = W_out[0]

    import ml_dtypes
    in_maps = []
    for m in range(NCORES):
        sl = slice(m * BL, (m + 1) * BL)
        z_aug = np.empty((P, I, BL), dtype=f)
        z_aug[:, 0, :] = 0.0
        z_aug[0, 0, :] = x[sl, -1, 0]
        z_aug[:, 1:-1, :] = np.transpose(z[sl, T - P :, :], (1, 2, 0))
        z_aug[:, -1, :] = 1.0
        in_maps.append(
            {
                "zt": np.ascontiguousarray(z_aug),
                "h0t": np.ascontiguousarray(h0[0, sl, :].T).astype(
                    ml_dtypes.bfloat16),
                "c0t": np.ascontiguousarray(c0[0, sl, :].T).astype(ml_dtypes.bfloat16),
                "wz0t": wz0t,
                "wzft": wzft,
                "whh0t": whh0t.astype(ml_dtypes.bfloat16),
                "whhft": whhft.astype(ml_dtypes.bfloat16),
                "woutt": woutt.astype(ml_dtypes.bfloat16),
            }
        )
    return in_maps


def run_on_cores(inputs: dict, **spmd_kwargs):
    """Build + run; returns (full_output, BassKernelResults)."""
    inputs = {k: np.asarray(v, dtype=np.float32) for k, v in inputs.items()}
    nc = _get_nc()
    in_maps = _prep_in_maps(**inputs)
    res = run_bass_kernel_spmd(nc, in_maps, core_ids=list(range(NCORES)), **spmd_kwargs)
    outs = np.concatenate(
        [r["out"].T for r in res.results], axis=0
    )  # (8192, 64)
    outs = outs + np.float32(inputs["b_out"][0])
    return outs[:, :, None].astype(np.float32), res


def kernel(**inputs) -> np.ndarray:
    out, _ = run_on_cores(inputs)
    return out


# revision 7
# speedup vs baseline: 1.2997x; 1.0102x over previous
"""Trainium2 Bass kernel for nn_DecoderRNN (autoregressive LSTM decoder).

Strategy:
  - Pure data parallelism: batch 8192 -> 1024 per core across 8 NeuronCores.
  - Feature-major layout on chip: h^T, c^T are [H=128 partitions, B_local].
    gates^T = W^T blocks (stationary) @ activations (moving), so the
    elementwise LSTM cell update produces h^T directly in the layout the
    next step's matmul needs -- no per-step transposes.
  - Output projection folded into the recurrent weights (W~_hh = W_hh +
    W_ih[:,0:1] @ W_out); biases ride in the matmul via a ones-row in the
    z tile. Step 0 uses unfolded weights with x supplied via the z tile.
  - ACT (scalar engine) is the bottleneck: 5 transcendental passes over
    [128,1024] per step = 4.27us/step of column time at 1.2GHz, plus
    ~185ns fixed cost per activation instruction. The schedule targets
    exactly 8 ACT ops/step (sFI, G, sO, ct per half) in the stream order
      sFI0 G0 sO0 sFI1 G1 ct0 sO1 ct1
    which keeps every op's inputs ready >= its slot start and closes the
    per-half recurrence cycle (evict -> cell -> tanh(c) -> h -> h-side
    matmul -> next evict) in exactly the 6.0us/step ACT busy time.
  - Gate order [f, i, g, o]: f,i share a 2-bank PSUM tile (one sigmoid
    evict); g and o are 1-bank tiles. tanh(c) is one [128,512] op per
    half (not quartered: fewer ACT ops wins over finer pipelining).
  - Gate evictions write bf16: i*g runs in the DVE 2x packed mode, and
    h = o*tanh(c) is all-bf16 (h feeds the matmuls as bf16 moving data,
    same 1 cycle/row as f32r). c and f*c stay fp32 for accuracy.
  - All three cell ops (f*c, i*g, add) run on DVE -- GPSIMD's 0.42x
    multiply efficiency makes it too slow for the critical path; it only
    gets the out-row PSUM->SBUF copies.
  - z-side matmuls for step t+1 are pre-issued during step t; h-side
    matmuls are emitted f,i,g,o so the sFI evict unblocks after two mms.
  - Out rows: step t / half b lands on PSUM partition 32*b + t%32 via
    shifted W_out column blocks, so 64 rows accumulate in one PSUM bank
    and evict once per 32 steps.
  - PSUM budget (8 banks): FI 2bufs x 2 + G 2bufs x 1 + O 1buf x 1 +
    po32 1 = 8.
"""

import os
import sys

for _p in ("/opt/trn_rl_repo", "/root/.axon_site/_ro/trn_rl_repo"):
    if os.path.isdir(_p) and _p not in sys.path:
        sys.path.insert(0, _p)

from contextlib import ExitStack

import numpy as np

import concourse.bass as bass  # noqa: F401  (registers types)
import concourse.mybir as mybir
import concourse.tile as tile
from concourse import bacc
from concourse.bass_utils import run_bass_kernel_spmd

NCORES = 8
B, T, F, H, P = 8192, 128, 63, 128, 64
BL = B // NCORES      # 1024 rows per core
I = 2 + F             # 64 LSTM input features + 1 ones-row for bias
G4 = 4 * H            # 512 gate rows
NH = 2                # batch halves (moving-dim chunks of 512)
NW = BL // NH         # 512

_f32 = mybir.dt.float32
_f32r = mybir.dt.float32r
_bf16 = mybir.dt.bfloat16

_CACHE: dict = {}


def _build():
    nc = bacc.Bacc("TRN2", target_bir_lowering=False, debug=False)
    AF = mybir.ActivationFunctionType

    zt_d = nc.dram_tensor("zt", [P, I, BL], _f32r, kind="ExternalInput")
    h0_d = nc.dram_tensor("h0t", [H, BL], _bf16, kind="ExternalInput")
    c0_d = nc.dram_tensor("c0t", [H, BL], _bf16, kind="ExternalInput")
    # weight layouts: columns are gate rows permuted to [f, i, g, o]
    wz0_d = nc.dram_tensor("wz0t", [I, G4], _f32r, kind="ExternalInput")
    wzf_d = nc.dram_tensor("wzft", [I, G4], _f32r, kind="ExternalInput")
    wh0_d = nc.dram_tensor("whh0t", [H, G4], _bf16, kind="ExternalInput")
    whf_d = nc.dram_tensor("whhft", [H, G4], _bf16, kind="ExternalInput")
    wo_d = nc.dram_tensor("woutt", [H, H], _bf16, kind="ExternalInput")
    out_d = nc.dram_tensor("out", [P, BL], _f32, kind="ExternalOutput")

    with ExitStack() as ctx:
        tc = ctx.enter_context(tile.TileContext(nc))
        const = ctx.enter_context(tc.tile_pool(name="const", bufs=1))
        zp = ctx.enter_context(tc.tile_pool(name="z", bufs=4))
        hp = ctx.enter_context(tc.tile_pool(name="h", bufs=2))
        cp = ctx.enter_context(tc.tile_pool(name="c", bufs=2))
        gp = ctx.enter_context(tc.tile_pool(name="g", bufs=3))
        tp = ctx.enter_context(tc.tile_pool(name="t", bufs=3))
        op = ctx.enter_context(tc.tile_pool(name="osb", bufs=3))
        # PSUM budget (8 banks): FI 2x2 + G 2x1 + O 1x1 + po32 1 = 8
        psfi = ctx.enter_context(tc.tile_pool(name="psfi", bufs=2, space="PSUM"))
        psg = ctx.enter_context(tc.tile_pool(name="psg", bufs=2, space="PSUM"))
        pso = ctx.enter_context(tc.tile_pool(name="pso", bufs=1, space="PSUM"))
        pspo = ctx.enter_context(tc.tile_pool(name="pspo", bufs=1, space="PSUM"))

        # step-0-critical tensors first so the pipeline fills ASAP
        wz0 = const.tile([I, G4], _f32r, tag="wz0")
        nc.sync.dma_start(wz0[:], wz0_d[:])
        zt0 = zp.tile([I, BL], _f32r, tag="z", name="z0")
        nc.sync.dma_start(zt0[:, 0:NW], zt_d[0, :, 0:NW])
        nc.sync.dma_start(zt0[:, NW:BL], zt_d[0, :, NW:BL])
        h_prev = hp.tile([H, BL], _bf16, tag="h")
        nc.sync.dma_start(h_prev[:, 0:NW], h0_d[:, 0:NW])
        nc.sync.dma_start(h_prev[:, NW:BL], h0_d[:, NW:BL])
        wh0 = const.tile([H, G4], _bf16, tag="wh0")
        nc.sync.dma_start(wh0[:], wh0_d[:])
        c_prev = cp.tile([H, BL], _bf16, tag="c")
        nc.sync.dma_start(c_prev[:], c0_d[:])
        wzf = const.tile([I, G4], _f32r, tag="wzf")
        nc.sync.dma_start(wzf[:], wzf_d[:])
        whf = const.tile([H, G4], _bf16, tag="whf")
        nc.sync.dma_start(whf[:], whf_d[:])
        wo = const.tile([H, H], _bf16, tag="wo")
        nc.sync.dma_start(wo[:], wo_d[:])

        def z_mms(t, zt, ps):
            """z-side (and bias) matmul contributions for step t; emitted
            during step t-1, they run while the PE waits for h_t."""
            wz = wz0 if t == 0 else wzf
            nq = 2 if t == 0 else 1
            qw = NW // nq
            for half in range(NH):
                psFI = psfi.tile([H, 2 * NW], _f32, tag="fi",
                                 name=f"psFI{t}_{half}")
                psG = psg.tile([H, NW], _f32, tag="g", name=f"psG{t}_{half}")
                psO = pso.tile([H, NW], _f32, tag="o", name=f"psO{t}_{half}")
                ps[(t, half)] = (psFI, psG, psO)
                for j in range(2):      # f, i blocks
                    for q in range(nq):
                        js = slice(j * NW + q * qw, j * NW + (q + 1) * qw)
                        qs = slice(half * NW + q * qw, half * NW + (q + 1) * qw)
                        nc.tensor.matmul(psFI[:, js], wz[:, j * H : (j + 1) * H],
                                         zt[:, qs], start=(q == 0), stop=False)
                for q in range(nq):
                    qs = slice(half * NW + q * qw, half * NW + (q + 1) * qw)
                    qj = slice(q * qw, (q + 1) * qw)
                    nc.tensor.matmul(psG[:, qj], wz[:, 2 * H : 3 * H], zt[:, qs],
                                     start=(q == 0), stop=False)
                    nc.tensor.matmul(psO[:, qj], wz[:, 3 * H : 4 * H], zt[:, qs],
                                     start=(q == 0), stop=False)

        ps: dict = {}
        z_mms(0, zt0, ps)

        po32: dict = {}

        _PO_GROUPS = {}
        for _g0, _glen in ((0, 32), (32, 31), (63, 1)):
            for _t in range(_g0, _g0 + _glen):
                _PO_GROUPS[_t] = (_g0, _glen)

        def emit_po(tp_, h_tile):
            g0, glen = _PO_GROUPS[tp_]
            j = tp_ - g0
            if j == 0:
                po32[0] = pspo.tile([64, NW], _f32, tag="po32",
                                    name=f"po32_{tp_}")
            for half in range(NH):
                cs = slice(half * NW, (half + 1) * NW)
                blk = 63 - (half * 32 + j)
                nc.tensor.matmul(po32[0][:], wo[:, blk : blk + 64],
                                 h_tile[:, cs],
                                 start=(j == 0 and half == 0),
                                 stop=(j == glen - 1 and half == NH - 1))
            if j == glen - 1:
                orow32 = op.tile([64, NW], _f32, tag="orow", name=f"orow{tp_}")
                nc.vector.tensor_copy(orow32[:], po32[0][:])
                if glen == 1:
                    nc.sync.dma_start(out_d[g0 : g0 + 1, :],
                                      orow32[0:64:32, :])
                else:
                    for half in range(NH):
                        cs = slice(half * NW, (half + 1) * NW)
                        nc.sync.dma_start(out_d[g0 : g0 + glen, cs],
                                          orow32[32 * half : 32 * half + glen, :])

        def h_mms(t, half, psFI, psG, psO):
            """h-side matmuls: f,i in 256-col quarters (h lands in quarter
            chunks from the split h-mul, so the first mms start early and
            the sFI evict unblocks sooner), then g, o full-width."""
            wh = wh0 if t == 0 else whf
            cs = slice(half * NW, (half + 1) * NW)
            hw_ = NW // 2
            for q in range(2):
                for j in range(2):
                    js = slice(j * NW + q * hw_, j * NW + (q + 1) * hw_)
                    qs = slice(half * NW + q * hw_, half * NW + (q + 1) * hw_)
                    nc.tensor.matmul(psFI[:, js], wh[:, j * H : (j + 1) * H],
                                     h_prev[:, qs], start=False, stop=True)
            nc.tensor.matmul(psG[:], wh[:, 2 * H : 3 * H], h_prev[:, cs],
                             start=False, stop=True)
            nc.tensor.matmul(psO[:], wh[:, 3 * H : 4 * H], h_prev[:, cs],
                             start=False, stop=True)

        prev = None  # (t, h_tile) pending out-projection
        for t in range(P):
            h_new = hp.tile([H, BL], _bf16, tag="h", name=f"h{t}")
            c_new = cp.tile([H, BL], _bf16, tag="c", name=f"c{t}")
            gFI = [None, None]
            gG = [None, None]
            gO = [None, None]
            ct = [None, None]

            def evict_fi(half):
                psFI, _, _ = ps[(t, half)]
                gFI[half] = gp.tile([H, 2 * NW], _bf16, tag="gFI",
                                    name=f"gFI{t}_{half}")
                nc.scalar.activation(gFI[half][:], psFI[:], AF.Sigmoid)

            def evict_g(half):
                _, psG, _ = ps[(t, half)]
                gG[half] = gp.tile([H, NW], _bf16, tag="gG",
                                   name=f"gG{t}_{half}")
                nc.scalar.activation(gG[half][:], psG[:], AF.Tanh)

            def evict_o(half):
                _, _, psO = ps[(t, half)]
                gO[half] = gp.tile([H, NW], _bf16, tag="gO",
                                   name=f"gO{t}_{half}")
                nc.scalar.activation(gO[half][:], psO[:], AF.Sigmoid)

            def cell(half):
                """c = f*c_prev + i*g, all on DVE (t1 fp32, t2 bf16 2x)."""
                cs = slice(half * NW, (half + 1) * NW)
                f_s = gFI[half][:, 0:NW]
                i_s = gFI[half][:, NW : 2 * NW]
                t1 = tp.tile([H, NW], _bf16, tag="t1", name=f"t1_{t}_{half}")
                nc.vector.tensor_mul(t1[:], f_s, c_prev[:, cs])
                t2 = tp.tile([H, NW], _bf16, tag="t2", name=f"t2_{t}_{half}")
                nc.vector.tensor_mul(t2[:], i_s, gG[half][:])
                nc.vector.tensor_add(c_new[:, cs], t1[:], t2[:])

            def tanh_c(half):
                cs = slice(half * NW, (half + 1) * NW)
                ct[half] = tp.tile([H, NW], _bf16, tag="ct",
                                   name=f"ct{t}_{half}")
                nc.scalar.activation(ct[half][:], c_new[:, cs], AF.Tanh)

            def h_mul(half):
                """h = o * tanh(c) in two 256-col chunks so the next step's
                f,i matmuls can start on the first chunk early."""
                hw_ = NW // 2
                for q in range(2):
                    qs = slice(half * NW + q * hw_, half * NW + (q + 1) * hw_)
                    qq = slice(q * hw_, (q + 1) * hw_)
                    nc.vector.tensor_mul(h_new[:, qs], gO[half][:, qq],
                                         ct[half][:, qq])

            # ACT priority order: sFI0 G0 sO0 sFI1 G1 ct0 sO1 ct1
            h_mms(t, 0, *ps[(t, 0)])
            evict_fi(0)
            evict_g(0)
            evict_o(0)
            cell(0)
            h_mms(t, 1, *ps[(t, 1)])
            evict_fi(1)
            evict_g(1)
            tanh_c(0)
            cell(1)
            h_mul(0)
            evict_o(1)
            tanh_c(1)
            h_mul(1)
            ps.pop((t, 0))
            ps.pop((t, 1))
            if t + 1 < P:
                zt = zp.tile([I, BL], _f32r, tag="z", name=f"z{t + 1}")
                nc.sync.dma_start(zt[:], zt_d[t + 1, :, :])
                z_mms(t + 1, zt, ps)
            if prev is not None:
                emit_po(prev[0], prev[1])
            prev = (t, h_new)
            h_prev, c_prev = h_new, c_new
        emit_po(prev[0], prev[1])

    nc.compile()
    return nc


def _get_nc():
    if "nc" not in _CACHE:
        _CACHE["nc"] = _build()
    return _CACHE["nc"]


# gate-row permutation: PyTorch order [i,f,g,o] -> kernel order [f,i,g,o]
_PERM = np.concatenate(
    [np.arange(H, 2 * H), np.arange(0, H), np.arange(2 * H, 3 * H),
     np.arange(3 * H, 4 * H)]
)


def _prep_in_maps(x, z, h0, c0, W_ih, W_hh, b_ih, b_hh, W_out, b_out):
    f = np.float32
    Wihp = W_ih[_PERM]                                   # (512, 64)
    Whhp = W_hh[_PERM]                                   # (512, 128)
    Whfp = Whhp + Wihp[:, 0:1] @ W_out                   # fold out-projection
    b0 = (b_ih + b_hh)[_PERM].astype(f)
    bf = (b0 + Wihp[:, 0] * b_out[0]).astype(f)

    wz0t = np.concatenate([Wihp.T, b0[None, :]], axis=0).astype(f)   # (65, 512)
    wzft = np.concatenate([Wihp.T, bf[None, :]], axis=0).astype(f)   # (65, 512)
    whh0t = np.ascontiguousarray(Whhp.T, dtype=f)                    # (128, 512)
    whhft = np.ascontiguousarray(Whfp.T, dtype=f)                    # (128, 512)
    woutt = np.zeros((H, H), dtype=f)
    woutt[:, 63] = W_out[0]

    import ml_dtypes
    in_maps = []
    for m in range(NCORES):
        sl = slice(m * BL, (m + 1) * BL)
        z_aug = np.empty((P, I, BL), dtype=f)
        z_aug[:, 0, :] = 0.0
        z_aug[0, 0, :] = x[sl, -1, 0]
        z_aug[:, 1:-1, :] = np.transpose(z[sl, T - P :, :], (1, 2, 0))
        z_aug[:, -1, :] = 1.0
        in_maps.append(
            {
                "zt": np.ascontiguousarray(z_aug),
                "h0t": np.ascontiguousarray(h0[0, sl, :].T).astype(
                    ml_dtypes.bfloat16),
                "c0t": np.ascontiguousarray(c0[0, sl, :].T).astype(ml_dtypes.bfloat16),
                "wz0t": wz0t,
                "wzft": wzft,
                "whh0t": whh0t.astype(ml_dtypes.bfloat16),
                "whhft": whhft.astype(ml_dtypes.bfloat16),
                "woutt": woutt.astype(ml_dtypes.bfloat16),
            }
        )
    return in_maps


def run_on_cores(inputs: dict, **spmd_kwargs):
    """Build + run; returns (full_output, BassKernelResults)."""
    inputs = {k: np.asarray(v, dtype=np.float32) for k, v in inputs.items()}
    nc = _get_nc()
    in_maps = _prep_in_maps(**inputs)
    res = run_bass_kernel_spmd(nc, in_maps, core_ids=list(range(NCORES)), **spmd_kwargs)
    outs = np.concatenate(
        [r["out"].T for r in res.results], axis=0
    )  # (8192, 64)
    outs = outs + np.float32(inputs["b_out"][0])
    return outs[:, :, None].astype(np.float32), res


def kernel(**inputs) -> np.ndarray:
    out, _ = run_on_cores(inputs)
    return out


# revision 9
# speedup vs baseline: 1.3006x; 1.0007x over previous
"""Trainium2 Bass kernel for nn_DecoderRNN (autoregressive LSTM decoder).

Strategy:
  - Pure data parallelism: batch 8192 -> 1024 per core across 8 NeuronCores.
  - Feature-major layout on chip: h^T, c^T are [H=128 partitions, B_local].
    gates^T = W^T blocks (stationary) @ activations (moving), so the
    elementwise LSTM cell update produces h^T directly in the layout the
    next step's matmul needs -- no per-step transposes.
  - Output projection folded into the recurrent weights (W~_hh = W_hh +
    W_ih[:,0:1] @ W_out); biases ride in the matmul via a ones-row in the
    z tile. Step 0 uses unfolded weights with x supplied via the z tile.
  - ACT (scalar engine) is the bottleneck: 5 transcendental passes over
    [128,1024] per step = 4.27us/step of column time at 1.2GHz, plus
    ~185ns fixed cost per activation instruction. The schedule targets
    exactly 8 ACT ops/step (sFI, G, sO, ct per half) in the stream order
      sFI0 G0 sO0 sFI1 G1 ct0 sO1 ct1
    which keeps every op's inputs ready >= its slot start and closes the
    per-half recurrence cycle (evict -> cell -> tanh(c) -> h -> h-side
    matmul -> next evict) in exactly the 6.0us/step ACT busy time.
  - Gate order [f, i, g, o]: f,i share a 2-bank PSUM tile (one sigmoid
    evict); g and o are 1-bank tiles. tanh(c) is one [128,512] op per
    half (not quartered: fewer ACT ops wins over finer pipelining).
  - Gate evictions write bf16: i*g runs in the DVE 2x packed mode, and
    h = o*tanh(c) is all-bf16 (h feeds the matmuls as bf16 moving data,
    same 1 cycle/row as f32r). c and f*c stay fp32 for accuracy.
  - All three cell ops (f*c, i*g, add) run on DVE -- GPSIMD's 0.42x
    multiply efficiency makes it too slow for the critical path; it only
    gets the out-row PSUM->SBUF copies.
  - z-side matmuls for step t+1 are pre-issued during step t; h-side
    matmuls are emitted f,i,g,o so the sFI evict unblocks after two mms.
  - Out rows: step t / half b lands on PSUM partition 32*b + t%32 via
    shifted W_out column blocks, so 64 rows accumulate in one PSUM bank
    and evict once per 32 steps.
  - PSUM budget (8 banks): FI 2bufs x 2 + G 2bufs x 1 + O 1buf x 1 +
    po32 1 = 8.
"""

import os
import sys

for _p in ("/opt/trn_rl_repo", "/root/.axon_site/_ro/trn_rl_repo"):
    if os.path.isdir(_p) and _p not in sys.path:
        sys.path.insert(0, _p)

from contextlib import ExitStack

import numpy as np

import concourse.bass as bass  # noqa: F401  (registers types)
import concourse.mybir as mybir
import concourse.tile as tile
from concourse import bacc
from concourse.bass_utils import run_bass_kernel_spmd

NCORES = 8
B, T, F, H, P = 8192, 128, 63, 128, 64
BL = B // NCORES      # 1024 rows per core
I = 2 + F             # 64 LSTM input features + 1 ones-row for bias
G4 = 4 * H            # 512 gate rows
NH = 2                # batch halves (moving-dim chunks of 512)
NW = BL // NH         # 512

_f32 = mybir.dt.float32
_f32r = mybir.dt.float32r
_bf16 = mybir.dt.bfloat16

_CACHE: dict = {}


def _build():
    nc = bacc.Bacc("TRN2", target_bir_lowering=False, debug=False)
    AF = mybir.ActivationFunctionType

    zt_d = nc.dram_tensor("zt", [P, I, BL], _f32r, kind="ExternalInput")
    # packed inputs: fewer DMAs -> less HWDGE serialization in the prologue.
    # hc0: [h0 | c0] bf16; wz: [wz0 | wzf] f32r; wh: [wh0 | whf | wo] bf16.
    # weight layouts: columns are gate rows permuted to [f, i, g, o]
    hc0_d = nc.dram_tensor("hc0t", [H, 2 * BL], _bf16, kind="ExternalInput")
    wz_d = nc.dram_tensor("wzt", [I, 2 * G4], _f32r, kind="ExternalInput")
    wh_d = nc.dram_tensor("wht", [H, 2 * G4 + H], _bf16, kind="ExternalInput")
    out_d = nc.dram_tensor("out", [P, BL], _f32, kind="ExternalOutput")

    with ExitStack() as ctx:
        tc = ctx.enter_context(tile.TileContext(nc))
        const = ctx.enter_context(tc.tile_pool(name="const", bufs=1))
        zp = ctx.enter_context(tc.tile_pool(name="z", bufs=4))
        hp = ctx.enter_context(tc.tile_pool(name="h", bufs=2))
        cp = ctx.enter_context(tc.tile_pool(name="c", bufs=2))
        gp = ctx.enter_context(tc.tile_pool(name="g", bufs=3))
        tp = ctx.enter_context(tc.tile_pool(name="t", bufs=3))
        op = ctx.enter_context(tc.tile_pool(name="osb", bufs=3))
        # PSUM budget (8 banks): FI 2x2 + G 2x1 + O 1x1 + po32 1 = 8
        psfi = ctx.enter_context(tc.tile_pool(name="psfi", bufs=2, space="PSUM"))
        psg = ctx.enter_context(tc.tile_pool(name="psg", bufs=2, space="PSUM"))
        pso = ctx.enter_context(tc.tile_pool(name="pso", bufs=1, space="PSUM"))
        pspo = ctx.enter_context(tc.tile_pool(name="pspo", bufs=1, space="PSUM"))

        # step-0-critical tensors first so the pipeline fills ASAP; packed
        # transfers keep the HWDGE queue short.
        wzt = const.tile([I, 2 * G4], _f32r, tag="wzt")
        nc.sync.dma_start(wzt[:], wz_d[:])
        wz0 = wzt[:, 0:G4]
        wzf = wzt[:, G4 : 2 * G4]
        zt0 = zp.tile([I, BL], _f32r, tag="z", name="z0")
        nc.sync.dma_start(zt0[:], zt_d[0, :, :])
        hc0 = const.tile([H, 2 * BL], _bf16, tag="hc0")
        nc.sync.dma_start(hc0[:], hc0_d[:])
        h_prev = hc0[:, 0:BL]
        c_prev = hc0[:, BL : 2 * BL]
        wht = const.tile([H, 2 * G4 + H], _bf16, tag="wht")
        nc.sync.dma_start(wht[:], wh_d[:])
        wh0 = wht[:, 0:G4]
        whf = wht[:, G4 : 2 * G4]
        wo = wht[:, 2 * G4 : 2 * G4 + H]

        def z_mms(t, zt, ps):
            """z-side (and bias) matmul contributions for step t; emitted
            during step t-1, they run while the PE waits for h_t. For t=0
            the matching h-side matmuls are interleaved per gate block so
            the first sigmoid evict isn't stuck behind all 16 z-matmuls on
            the cold (p-state-throttled) PE."""
            wz = wz0 if t == 0 else wzf
            nq = 2 if t == 0 else 1
            qw = NW // nq
            for half in range(NH):
                psFI = psfi.tile([H, 2 * NW], _f32, tag="fi",
                                 name=f"psFI{t}_{half}")
                psG = psg.tile([H, NW], _f32, tag="g", name=f"psG{t}_{half}")
                psO = pso.tile([H, NW], _f32, tag="o", name=f"psO{t}_{half}")
                ps[(t, half)] = (psFI, psG, psO)
                for j in range(2):      # f, i blocks
                    for q in range(nq):
                        js = slice(j * NW + q * qw, j * NW + (q + 1) * qw)
                        qs = slice(half * NW + q * qw, half * NW + (q + 1) * qw)
                        nc.tensor.matmul(psFI[:, js], wz[:, j * H : (j + 1) * H],
                                         zt[:, qs], start=(q == 0), stop=False)
                if t == 0:
                    h_mms_fi(t, half, psFI)
                for q in range(nq):
                    qs = slice(half * NW + q * qw, half * NW + (q + 1) * qw)
                    qj = slice(q * qw, (q + 1) * qw)
                    nc.tensor.matmul(psG[:, qj], wz[:, 2 * H : 3 * H], zt[:, qs],
                                     start=(q == 0), stop=False)
                    nc.tensor.matmul(psO[:, qj], wz[:, 3 * H : 4 * H], zt[:, qs],
                                     start=(q == 0), stop=False)
                if t == 0:
                    h_mms_go(t, half, psG, psO)

        def h_mms_fi(t, half, psFI):
            """h-side f,i matmuls in 256-col quarters (h lands in quarter
            chunks from the split h-mul, so the first mms start early and
            the sFI evict unblocks sooner)."""
            wh = wh0 if t == 0 else whf
            hw_ = NW // 2
            for q in range(2):
                for j in range(2):
                    js = slice(j * NW + q * hw_, j * NW + (q + 1) * hw_)
                    qs = slice(half * NW + q * hw_, half * NW + (q + 1) * hw_)
                    nc.tensor.matmul(psFI[:, js], wh[:, j * H : (j + 1) * H],
                                     h_prev[:, qs], start=False, stop=True)

        def h_mms_go(t, half, psG, psO):
            wh = wh0 if t == 0 else whf
            cs = slice(half * NW, (half + 1) * NW)
            nc.tensor.matmul(psG[:], wh[:, 2 * H : 3 * H], h_prev[:, cs],
                             start=False, stop=True)
            nc.tensor.matmul(psO[:], wh[:, 3 * H : 4 * H], h_prev[:, cs],
                             start=False, stop=True)

        ps: dict = {}
        z_mms(0, zt0, ps)

        po32: dict = {}

        _PO_GROUPS = {}
        for _g0, _glen in ((0, 32), (32, 31), (63, 1)):
            for _t in range(_g0, _g0 + _glen):
                _PO_GROUPS[_t] = (_g0, _glen)

        def emit_po(tp_, h_tile):
            g0, glen = _PO_GROUPS[tp_]
            j = tp_ - g0
            if j == 0:
                po32[0] = pspo.tile([64, NW], _f32, tag="po32",
                                    name=f"po32_{tp_}")
            for half in range(NH):
                cs = slice(half * NW, (half + 1) * NW)
                blk = 63 - (half * 32 + j)
                nc.tensor.matmul(po32[0][:], wo[:, blk : blk + 64],
                                 h_tile[:, cs],
                                 start=(j == 0 and half == 0),
                                 stop=(j == glen - 1 and half == NH - 1))
            if j == glen - 1:
                orow32 = op.tile([64, NW], _f32, tag="orow", name=f"orow{tp_}")
                nc.vector.tensor_copy(orow32[:], po32[0][:])
                if glen == 1:
                    nc.sync.dma_start(out_d[g0 : g0 + 1, :],
                                      orow32[0:64:32, :])
                else:
                    for half in range(NH):
                        cs = slice(half * NW, (half + 1) * NW)
                        nc.sync.dma_start(out_d[g0 : g0 + glen, cs],
                                          orow32[32 * half : 32 * half + glen, :])

        prev = None  # (t, h_tile) pending out-projection
        for t in range(P):
            h_new = hp.tile([H, BL], _bf16, tag="h", name=f"h{t}")
            c_new = cp.tile([H, BL], _bf16, tag="c", name=f"c{t}")
            gFI = [None, None]
            gG = [None, None]
            gO = [None, None]
            ct = [None, None]

            def evict_fi(half):
                psFI, _, _ = ps[(t, half)]
                gFI[half] = gp.tile([H, 2 * NW], _bf16, tag="gFI",
                                    name=f"gFI{t}_{half}")
                nc.scalar.activation(gFI[half][:], psFI[:], AF.Sigmoid)

            def evict_g(half):
                _, psG, _ = ps[(t, half)]
                gG[half] = gp.tile([H, NW], _bf16, tag="gG",
                                   name=f"gG{t}_{half}")
                nc.scalar.activation(gG[half][:], psG[:], AF.Tanh)

            def evict_o(half):
                _, _, psO = ps[(t, half)]
                gO[half] = gp.tile([H, NW], _bf16, tag="gO",
                                   name=f"gO{t}_{half}")
                nc.scalar.activation(gO[half][:], psO[:], AF.Sigmoid)

            def cell(half):
                """c = f*c_prev + i*g, all on DVE (t1 fp32, t2 bf16 2x)."""
                cs = slice(half * NW, (half + 1) * NW)
                f_s = gFI[half][:, 0:NW]
                i_s = gFI[half][:, NW : 2 * NW]
                t1 = tp.tile([H, NW], _bf16, tag="t1", name=f"t1_{t}_{half}")
                nc.vector.tensor_mul(t1[:], f_s, c_prev[:, cs])
                t2 = tp.tile([H, NW], _bf16, tag="t2", name=f"t2_{t}_{half}")
                nc.vector.tensor_mul(t2[:], i_s, gG[half][:])
                nc.vector.tensor_add(c_new[:, cs], t1[:], t2[:])

            def tanh_c(half):
                cs = slice(half * NW, (half + 1) * NW)
                ct[half] = tp.tile([H, NW], _bf16, tag="ct",
                                   name=f"ct{t}_{half}")
                nc.scalar.activation(ct[half][:], c_new[:, cs], AF.Tanh)

            def h_mul(half):
                """h = o * tanh(c) in two 256-col chunks so the next step's
                f,i matmuls can start on the first chunk early."""
                hw_ = NW // 2
                for q in range(2):
                    qs = slice(half * NW + q * hw_, half * NW + (q + 1) * hw_)
                    qq = slice(q * hw_, (q + 1) * hw_)
                    nc.vector.tensor_mul(h_new[:, qs], gO[half][:, qq],
                                         ct[half][:, qq])

            # ACT priority order: sFI0 G0 sO0 sFI1 G1 ct0 sO1 ct1
            if t > 0:
                psFI0, psG0, psO0 = ps[(t, 0)]
                h_mms_fi(t, 0, psFI0)
                h_mms_go(t, 0, psG0, psO0)
            evict_fi(0)
            evict_g(0)
            evict_o(0)
            cell(0)
            if t > 0:
                psFI1, psG1, psO1 = ps[(t, 1)]
                h_mms_fi(t, 1, psFI1)
                h_mms_go(t, 1, psG1, psO1)
            evict_fi(1)
            evict_g(1)
            tanh_c(0)
            cell(1)
            h_mul(0)
            evict_o(1)
            tanh_c(1)
            h_mul(1)
            ps.pop((t, 0))
            ps.pop((t, 1))
            if t + 1 < P:
                zt = zp.tile([I, BL], _f32r, tag="z", name=f"z{t + 1}")
                nc.sync.dma_start(zt[:], zt_d[t + 1, :, :])
                z_mms(t + 1, zt, ps)
            if prev is not None:
                emit_po(prev[0], prev[1])
            prev = (t, h_new)
            h_prev, c_prev = h_new, c_new
        emit_po(prev[0], prev[1])

    nc.compile()
    return nc


def _get_nc():
    if "nc" not in _CACHE:
        _CACHE["nc"] = _build()
    return _CACHE["nc"]


# gate-row permutation: PyTorch order [i,f,g,o] -> kernel order [f,i,g,o]
_PERM = np.concatenate(
    [np.arange(H, 2 * H), np.arange(0, H), np.arange(2 * H, 3 * H),
     np.arange(3 * H, 4 * H)]
)


def _prep_in_maps(x, z, h0, c0, W_ih, W_hh, b_ih, b_hh, W_out, b_out):
    f = np.float32
    Wihp = W_ih[_PERM]                                   # (512, 64)
    Whhp = W_hh[_PERM]                                   # (512, 128)
    Whfp = Whhp + Wihp[:, 0:1] @ W_out                   # fold out-projection
    b0 = (b_ih + b_hh)[_PERM].astype(f)
    bf = (b0 + Wihp[:, 0] * b_out[0]).astype(f)

    wz0t = np.concatenate([Wihp.T, b0[None, :]], axis=0).astype(f)   # (65, 512)
    wzft = np.concatenate([Wihp.T, bf[None, :]], axis=0).astype(f)   # (65, 512)
    whh0t = np.ascontiguousarray(Whhp.T, dtype=f)                    # (128, 512)
    whhft = np.ascontiguousarray(Whfp.T, dtype=f)                    # (128, 512)
    woutt = np.zeros((H, H), dtype=f)
    woutt[:, 63] = W_out[0]

    import ml_dtypes
    in_maps = []
    for m in range(NCORES):
        sl = slice(m * BL, (m + 1) * BL)
        z_aug = np.empty((P, I, BL), dtype=f)
        z_aug[:, 0, :] = 0.0
        z_aug[0, 0, :] = x[sl, -1, 0]
        z_aug[:, 1:-1, :] = np.transpose(z[sl, T - P :, :], (1, 2, 0))
        z_aug[:, -1, :] = 1.0
        hc0 = np.concatenate(
            [h0[0, sl, :].T, c0[0, sl, :].T], axis=1)           # (128, 2048)
        wht = np.concatenate([whh0t, whhft, woutt], axis=1)      # (128, 1152)
        in_maps.append(
            {
                "zt": np.ascontiguousarray(z_aug),
                "hc0t": np.ascontiguousarray(hc0).astype(ml_dtypes.bfloat16),
                "wzt": np.ascontiguousarray(
                    np.concatenate([wz0t, wzft], axis=1)),       # (65, 1024)
                "wht": np.ascontiguousarray(wht).astype(ml_dtypes.bfloat16),
            }
        )
    return in_maps


def run_on_cores(inputs: dict, **spmd_kwargs):
    """Build + run; returns (full_output, BassKernelResults)."""
    inputs = {k: np.asarray(v, dtype=np.float32) for k, v in inputs.items()}
    nc = _get_nc()
    in_maps = _prep_in_maps(**inputs)
    res = run_bass_kernel_spmd(nc, in_maps, core_ids=list(range(NCORES)), **spmd_kwargs)
    outs = np.concatenate(
        [r["out"].T for r in res.results], axis=0
    )  # (8192, 64)
    outs = outs + np.float32(inputs["b_out"][0])
    return outs[:, :, None].astype(np.float32), res


def kernel(**inputs) -> np.ndarray:
    out, _ = run_on_cores(inputs)
    return out


# revision 10
# speedup vs baseline: 1.3089x; 1.0063x over previous
"""Trainium2 Bass kernel for nn_DecoderRNN (autoregressive LSTM decoder).

Strategy:
  - Pure data parallelism: batch 8192 -> 1024 per core across 8 NeuronCores.
  - Feature-major layout on chip: h^T, c^T are [H=128 partitions, B_local].
    gates^T = W^T blocks (stationary) @ activations (moving), so the
    elementwise LSTM cell update produces h^T directly in the layout the
    next step's matmul needs -- no per-step transposes.
  - Output projection folded into the recurrent weights (W~_hh = W_hh +
    W_ih[:,0:1] @ W_out); biases ride in the matmul via a ones-row in the
    z tile. Step 0 uses unfolded weights with x supplied via the z tile.
  - ACT (scalar engine) is the bottleneck: 5 transcendental passes over
    [128,1024] per step = 4.27us/step of column time at 1.2GHz, plus
    ~185ns fixed cost per activation instruction. The schedule targets
    exactly 8 ACT ops/step (sFI, G, sO, ct per half) in the stream order
      sFI0 G0 sO0 sFI1 G1 ct0 sO1 ct1
    which keeps every op's inputs ready >= its slot start and closes the
    per-half recurrence cycle (evict -> cell -> tanh(c) -> h -> h-side
    matmul -> next evict) in exactly the 6.0us/step ACT busy time.
  - Gate order [f, i, g, o]: f,i share a 2-bank PSUM tile (one sigmoid
    evict); g and o are 1-bank tiles. tanh(c) is one [128,512] op per
    half (not quartered: fewer ACT ops wins over finer pipelining).
  - Gate evictions write bf16: i*g runs in the DVE 2x packed mode, and
    h = o*tanh(c) is all-bf16 (h feeds the matmuls as bf16 moving data,
    same 1 cycle/row as f32r). c and f*c stay fp32 for accuracy.
  - All three cell ops (f*c, i*g, add) run on DVE -- GPSIMD's 0.42x
    multiply efficiency makes it too slow for the critical path; it only
    gets the out-row PSUM->SBUF copies.
  - z-side matmuls for step t+1 are pre-issued during step t; h-side
    matmuls are emitted f,i,g,o so the sFI evict unblocks after two mms.
  - Out rows: step t / half b lands on PSUM partition 32*b + t%32 via
    shifted W_out column blocks, so 64 rows accumulate in one PSUM bank
    and evict once per 32 steps.
  - PSUM budget (8 banks): FI 2bufs x 2 + G 2bufs x 1 + O 1buf x 1 +
    po32 1 = 8.
"""

import os
import sys

for _p in ("/opt/trn_rl_repo", "/root/.axon_site/_ro/trn_rl_repo"):
    if os.path.isdir(_p) and _p not in sys.path:
        sys.path.insert(0, _p)

from contextlib import ExitStack

import numpy as np

import concourse.bass as bass  # noqa: F401  (registers types)
import concourse.mybir as mybir
import concourse.tile as tile
from concourse import bacc
from concourse.bass_utils import run_bass_kernel_spmd

NCORES = 8
B, T, F, H, P = 8192, 128, 63, 128, 64
BL = B // NCORES      # 1024 rows per core
I = 2 + F             # 64 LSTM input features + 1 ones-row for bias
G4 = 4 * H            # 512 gate rows
NH = 2                # batch halves (moving-dim chunks of 512)
NW = BL // NH         # 512

_f32 = mybir.dt.float32
_f32r = mybir.dt.float32r
_bf16 = mybir.dt.bfloat16

_CACHE: dict = {}


def _build():
    nc = bacc.Bacc("TRN2", target_bir_lowering=False, debug=False)
    AF = mybir.ActivationFunctionType

    zt_d = nc.dram_tensor("zt", [P, I, BL], _f32r, kind="ExternalInput")
    # packed inputs: fewer DMAs -> less HWDGE serialization in the prologue.
    # wzz: [wz0 | wzf | z_0] f32r; whc: [wh0 | whf | wo | h0 | c0] bf16.
    # weight layouts: columns are gate rows permuted to [f, i, g, o]
    wzz_d = nc.dram_tensor("wzzt", [I, 2 * G4 + BL], _f32r, kind="ExternalInput")
    whc_d = nc.dram_tensor("whct", [H, 2 * G4 + H + 2 * BL], _bf16,
                           kind="ExternalInput")
    out_d = nc.dram_tensor("out", [P, BL], _f32, kind="ExternalOutput")

    with ExitStack() as ctx:
        tc = ctx.enter_context(tile.TileContext(nc))
        const = ctx.enter_context(tc.tile_pool(name="const", bufs=1))
        zp = ctx.enter_context(tc.tile_pool(name="z", bufs=4))
        hp = ctx.enter_context(tc.tile_pool(name="h", bufs=2))
        cp = ctx.enter_context(tc.tile_pool(name="c", bufs=2))
        gp = ctx.enter_context(tc.tile_pool(name="g", bufs=3))
        tp = ctx.enter_context(tc.tile_pool(name="t", bufs=3))
        op = ctx.enter_context(tc.tile_pool(name="osb", bufs=3))
        # PSUM budget (8 banks): FI 2x2 + G 2x1 + O 1x1 + po32 1 = 8
        psfi = ctx.enter_context(tc.tile_pool(name="psfi", bufs=2, space="PSUM"))
        psg = ctx.enter_context(tc.tile_pool(name="psg", bufs=2, space="PSUM"))
        pso = ctx.enter_context(tc.tile_pool(name="pso", bufs=1, space="PSUM"))
        pspo = ctx.enter_context(tc.tile_pool(name="pspo", bufs=1, space="PSUM"))

        # PE warmup: the tensor engine's clock ramps with sustained use
        # (p-state model: 0.65 -> 1.2 -> 2.4 GHz after 3us busy). Dummy
        # matmuls during the input-DMA window mean the real step-0 matmuls
        # start at full clock instead of 1.54 ns/col.
        wrm = tp.tile([H, H], _bf16, tag="wrm")
        nc.vector.memset(wrm[:], 0.0)
        wps = pspo.tile([H, H], _f32, tag="po32", name="warmup_ps")
        for _w in range(44):
            nc.tensor.matmul(wps[:], wrm[:], wrm[:], start=True, stop=True)

        # step-0-critical tensors first so the pipeline fills ASAP; two
        # packed transfers keep the HWDGE queue short.
        wzzt = const.tile([I, 2 * G4 + BL], _f32r, tag="wzzt")
        nc.sync.dma_start(wzzt[:], wzz_d[:])
        wz0 = wzzt[:, 0:G4]
        wzf = wzzt[:, G4 : 2 * G4]
        zt0 = wzzt[:, 2 * G4 : 2 * G4 + BL]
        whct = const.tile([H, 2 * G4 + H + 2 * BL], _bf16, tag="whct")
        nc.sync.dma_start(whct[:], whc_d[:])
        wh0 = whct[:, 0:G4]
        whf = whct[:, G4 : 2 * G4]
        wo = whct[:, 2 * G4 : 2 * G4 + H]
        h_prev = whct[:, 2 * G4 + H : 2 * G4 + H + BL]
        c_prev = whct[:, 2 * G4 + H + BL : 2 * G4 + H + 2 * BL]

        def z_mms(t, zt, ps):
            """z-side (and bias) matmul contributions for step t; emitted
            during step t-1, they run while the PE waits for h_t. For t=0
            the matching h-side matmuls are interleaved per gate block so
            the first sigmoid evict isn't stuck behind all 16 z-matmuls on
            the cold (p-state-throttled) PE."""
            wz = wz0 if t == 0 else wzf
            nq = 2 if t == 0 else 1
            qw = NW // nq
            for half in range(NH):
                psFI = psfi.tile([H, 2 * NW], _f32, tag="fi",
                                 name=f"psFI{t}_{half}")
                psG = psg.tile([H, NW], _f32, tag="g", name=f"psG{t}_{half}")
                psO = pso.tile([H, NW], _f32, tag="o", name=f"psO{t}_{half}")
                ps[(t, half)] = (psFI, psG, psO)
                for j in range(2):      # f, i blocks
                    for q in range(nq):
                        js = slice(j * NW + q * qw, j * NW + (q + 1) * qw)
                        qs = slice(half * NW + q * qw, half * NW + (q + 1) * qw)
                        nc.tensor.matmul(psFI[:, js], wz[:, j * H : (j + 1) * H],
                                         zt[:, qs], start=(q == 0), stop=False)
                if t == 0:
                    h_mms_fi(t, half, psFI)
                for q in range(nq):
                    qs = slice(half * NW + q * qw, half * NW + (q + 1) * qw)
                    qj = slice(q * qw, (q + 1) * qw)
                    nc.tensor.matmul(psG[:, qj], wz[:, 2 * H : 3 * H], zt[:, qs],
                                     start=(q == 0), stop=False)
                    nc.tensor.matmul(psO[:, qj], wz[:, 3 * H : 4 * H], zt[:, qs],
                                     start=(q == 0), stop=False)
                if t == 0:
                    h_mms_go(t, half, psG, psO)

        def h_mms_fi(t, half, psFI):
            """h-side f,i matmuls in 256-col quarters (h lands in quarter
            chunks from the split h-mul, so the first mms start early and
            the sFI evict unblocks sooner)."""
            wh = wh0 if t == 0 else whf
            hw_ = NW // 2
            for q in range(2):
                for j in range(2):
                    js = slice(j * NW + q * hw_, j * NW + (q + 1) * hw_)
                    qs = slice(half * NW + q * hw_, half * NW + (q + 1) * hw_)
                    nc.tensor.matmul(psFI[:, js], wh[:, j * H : (j + 1) * H],
                                     h_prev[:, qs], start=False, stop=True)

        def h_mms_go(t, half, psG, psO):
            wh = wh0 if t == 0 else whf
            cs = slice(half * NW, (half + 1) * NW)
            nc.tensor.matmul(psG[:], wh[:, 2 * H : 3 * H], h_prev[:, cs],
                             start=False, stop=True)
            nc.tensor.matmul(psO[:], wh[:, 3 * H : 4 * H], h_prev[:, cs],
                             start=False, stop=True)

        ps: dict = {}
        z_mms(0, zt0, ps)

        po32: dict = {}

        _PO_GROUPS = {}
        for _g0, _glen in ((0, 32), (32, 31), (63, 1)):
            for _t in range(_g0, _g0 + _glen):
                _PO_GROUPS[_t] = (_g0, _glen)

        def emit_po(tp_, h_tile):
            g0, glen = _PO_GROUPS[tp_]
            j = tp_ - g0
            if j == 0:
                po32[0] = pspo.tile([64, NW], _f32, tag="po32",
                                    name=f"po32_{tp_}")
            for half in range(NH):
                cs = slice(half * NW, (half + 1) * NW)
                blk = 63 - (half * 32 + j)
                nc.tensor.matmul(po32[0][:], wo[:, blk : blk + 64],
                                 h_tile[:, cs],
                                 start=(j == 0 and half == 0),
                                 stop=(j == glen - 1 and half == NH - 1))
            if j == glen - 1:
                orow32 = op.tile([64, NW], _f32, tag="orow", name=f"orow{tp_}")
                nc.vector.tensor_copy(orow32[:], po32[0][:])
                if glen == 1:
                    nc.sync.dma_start(out_d[g0 : g0 + 1, :],
                                      orow32[0:64:32, :])
                else:
                    for half in range(NH):
                        cs = slice(half * NW, (half + 1) * NW)
                        nc.sync.dma_start(out_d[g0 : g0 + glen, cs],
                                          orow32[32 * half : 32 * half + glen, :])

        prev = None  # (t, h_tile) pending out-projection
        for t in range(P):
            h_new = hp.tile([H, BL], _bf16, tag="h", name=f"h{t}")
            c_new = cp.tile([H, BL], _bf16, tag="c", name=f"c{t}")
            gFI = [None, None]
            gG = [None, None]
            gO = [None, None]
            ct = [None, None]

            def evict_fi(half):
                psFI, _, _ = ps[(t, half)]
                gFI[half] = gp.tile([H, 2 * NW], _bf16, tag="gFI",
                                    name=f"gFI{t}_{half}")
                nc.scalar.activation(gFI[half][:], psFI[:], AF.Sigmoid)

            def evict_g(half):
                _, psG, _ = ps[(t, half)]
                gG[half] = gp.tile([H, NW], _bf16, tag="gG",
                                   name=f"gG{t}_{half}")
                nc.scalar.activation(gG[half][:], psG[:], AF.Tanh)

            def evict_o(half):
                _, _, psO = ps[(t, half)]
                gO[half] = gp.tile([H, NW], _bf16, tag="gO",
                                   name=f"gO{t}_{half}")
                nc.scalar.activation(gO[half][:], psO[:], AF.Sigmoid)

            def cell(half):
                """c = f*c_prev + i*g, all on DVE (t1 fp32, t2 bf16 2x)."""
                cs = slice(half * NW, (half + 1) * NW)
                f_s = gFI[half][:, 0:NW]
                i_s = gFI[half][:, NW : 2 * NW]
                t1 = tp.tile([H, NW], _bf16, tag="t1", name=f"t1_{t}_{half}")
                nc.vector.tensor_mul(t1[:], f_s, c_prev[:, cs])
                t2 = tp.tile([H, NW], _bf16, tag="t2", name=f"t2_{t}_{half}")
                nc.vector.tensor_mul(t2[:], i_s, gG[half][:])
                nc.vector.tensor_add(c_new[:, cs], t1[:], t2[:])

            def tanh_c(half):
                cs = slice(half * NW, (half + 1) * NW)
                ct[half] = tp.tile([H, NW], _bf16, tag="ct",
                                   name=f"ct{t}_{half}")
                nc.scalar.activation(ct[half][:], c_new[:, cs], AF.Tanh)

            def h_mul(half):
                """h = o * tanh(c) in two 256-col chunks so the next step's
                f,i matmuls can start on the first chunk early."""
                hw_ = NW // 2
                for q in range(2):
                    qs = slice(half * NW + q * hw_, half * NW + (q + 1) * hw_)
                    qq = slice(q * hw_, (q + 1) * hw_)
                    nc.vector.tensor_mul(h_new[:, qs], gO[half][:, qq],
                                         ct[half][:, qq])

            # ACT priority order: sFI0 G0 sO0 sFI1 G1 ct0 sO1 ct1
            if t > 0:
                psFI0, psG0, psO0 = ps[(t, 0)]
                h_mms_fi(t, 0, psFI0)
                h_mms_go(t, 0, psG0, psO0)
            evict_fi(0)
            evict_g(0)
            evict_o(0)
            cell(0)
            if t > 0:
                psFI1, psG1, psO1 = ps[(t, 1)]
                h_mms_fi(t, 1, psFI1)
                h_mms_go(t, 1, psG1, psO1)
            evict_fi(1)
            evict_g(1)
            tanh_c(0)
            cell(1)
            h_mul(0)
            evict_o(1)
            tanh_c(1)
            h_mul(1)
            ps.pop((t, 0))
            ps.pop((t, 1))
            if t + 1 < P:
                zt = zp.tile([I, BL], _f32r, tag="z", name=f"z{t + 1}")
                nc.sync.dma_start(zt[:], zt_d[t + 1, :, :])
                z_mms(t + 1, zt, ps)
            if prev is not None:
                emit_po(prev[0], prev[1])
            prev = (t, h_new)
            h_prev, c_prev = h_new, c_new
        emit_po(prev[0], prev[1])

    nc.compile()
    return nc


def _get_nc():
    if "nc" not in _CACHE:
        _CACHE["nc"] = _build()
    return _CACHE["nc"]


# gate-row permutation: PyTorch order [i,f,g,o] -> kernel order [f,i,g,o]
_PERM = np.concatenate(
    [np.arange(H, 2 * H), np.arange(0, H), np.arange(2 * H, 3 * H),
     np.arange(3 * H, 4 * H)]
)


def _prep_in_maps(x, z, h0, c0, W_ih, W_hh, b_ih, b_hh, W_out, b_out):
    f = np.float32
    Wihp = W_ih[_PERM]                                   # (512, 64)
    Whhp = W_hh[_PERM]                                   # (512, 128)
    Whfp = Whhp + Wihp[:, 0:1] @ W_out                   # fold out-projection
    b0 = (b_ih + b_hh)[_PERM].astype(f)
    bf = (b0 + Wihp[:, 0] * b_out[0]).astype(f)

    wz0t = np.concatenate([Wihp.T, b0[None, :]], axis=0).astype(f)   # (65, 512)
    wzft = np.concatenate([Wihp.T, bf[None, :]], axis=0).astype(f)   # (65, 512)
    whh0t = np.ascontiguousarray(Whhp.T, dtype=f)                    # (128, 512)
    whhft = np.ascontiguousarray(Whfp.T, dtype=f)                    # (128, 512)
    woutt = np.zeros((H, H), dtype=f)
    woutt[:, 63] = W_out[0]

    import ml_dtypes
    in_maps = []
    for m in range(NCORES):
        sl = slice(m * BL, (m + 1) * BL)
        z_aug = np.empty((P, I, BL), dtype=f)
        z_aug[:, 0, :] = 0.0
        z_aug[0, 0, :] = x[sl, -1, 0]
        z_aug[:, 1:-1, :] = np.transpose(z[sl, T - P :, :], (1, 2, 0))
        z_aug[:, -1, :] = 1.0
        whc = np.concatenate(
            [whh0t, whhft, woutt, h0[0, sl, :].T, c0[0, sl, :].T],
            axis=1)                                              # (128, 3200)
        wzz = np.concatenate([wz0t, wzft, z_aug[0]], axis=1)     # (65, 2048)
        in_maps.append(
            {
                "zt": np.ascontiguousarray(z_aug),
                "wzzt": np.ascontiguousarray(wzz),
                "whct": np.ascontiguousarray(whc).astype(ml_dtypes.bfloat16),
            }
        )
    return in_maps


def run_on_cores(inputs: dict, **spmd_kwargs):
    """Build + run; returns (full_output, BassKernelResults)."""
    inputs = {k: np.asarray(v, dtype=np.float32) for k, v in inputs.items()}
    nc = _get_nc()
    in_maps = _prep_in_maps(**inputs)
    res = run_bass_kernel_spmd(nc, in_maps, core_ids=list(range(NCORES)), **spmd_kwargs)
    outs = np.concatenate(
        [r["out"].T for r in res.results], axis=0
    )  # (8192, 64)
    outs = outs + np.float32(inputs["b_out"][0])
    return outs[:, :, None].astype(np.float32), res


def kernel(**inputs) -> np.ndarray:
    out, _ = run_on_cores(inputs)
    return out


# revision 12
# speedup vs baseline: 1.3123x; 1.0026x over previous
"""Trainium2 Bass kernel for nn_DecoderRNN (autoregressive LSTM decoder).

Strategy:
  - Pure data parallelism: batch 8192 -> 1024 per core across 8 NeuronCores.
  - Feature-major layout on chip: h^T, c^T are [H=128 partitions, B_local].
    gates^T = W^T blocks (stationary) @ activations (moving), so the
    elementwise LSTM cell update produces h^T directly in the layout the
    next step's matmul needs -- no per-step transposes.
  - Output projection folded into the recurrent weights (W~_hh = W_hh +
    W_ih[:,0:1] @ W_out); biases ride in the matmul via a ones-row in the
    z tile. Step 0 uses unfolded weights with x supplied via the z tile.
  - ACT (scalar engine) is the bottleneck: 5 transcendental passes over
    [128,1024] per step = 4.27us/step of column time at 1.2GHz, plus
    ~185ns fixed cost per activation instruction. The schedule targets
    exactly 8 ACT ops/step (sFI, G, sO, ct per half) in the stream order
      sFI0 G0 sO0 sFI1 G1 ct0 sO1 ct1
    which keeps every op's inputs ready >= its slot start and closes the
    per-half recurrence cycle (evict -> cell -> tanh(c) -> h -> h-side
    matmul -> next evict) in exactly the 6.0us/step ACT busy time.
  - Gate order [f, i, g, o]: f,i share a 2-bank PSUM tile (one sigmoid
    evict); g and o are 1-bank tiles. tanh(c) is one [128,512] op per
    half (not quartered: fewer ACT ops wins over finer pipelining).
  - Gate evictions write bf16: i*g runs in the DVE 2x packed mode, and
    h = o*tanh(c) is all-bf16 (h feeds the matmuls as bf16 moving data,
    same 1 cycle/row as f32r). c and f*c stay fp32 for accuracy.
  - All three cell ops (f*c, i*g, add) run on DVE -- GPSIMD's 0.42x
    multiply efficiency makes it too slow for the critical path; it only
    gets the out-row PSUM->SBUF copies.
  - z-side matmuls for step t+1 are pre-issued during step t; h-side
    matmuls are emitted f,i,g,o so the sFI evict unblocks after two mms.
  - Out rows: step t / half b lands on PSUM partition 32*b + t%32 via
    shifted W_out column blocks, so 64 rows accumulate in one PSUM bank
    and evict once per 32 steps.
  - PSUM budget (8 banks): FI 2bufs x 2 + G 2bufs x 1 + O 1buf x 1 +
    po32 1 = 8.
"""

import os
import sys

for _p in ("/opt/trn_rl_repo", "/root/.axon_site/_ro/trn_rl_repo"):
    if os.path.isdir(_p) and _p not in sys.path:
        sys.path.insert(0, _p)

from contextlib import ExitStack

import numpy as np

import concourse.bass as bass  # noqa: F401  (registers types)
import concourse.mybir as mybir
import concourse.tile as tile
from concourse import bacc
from concourse.bass_utils import run_bass_kernel_spmd

NCORES = 8
B, T, F, H, P = 8192, 128, 63, 128, 64
BL = B // NCORES      # 1024 rows per core
I = 2 + F             # 64 LSTM input features + 1 ones-row for bias
G4 = 4 * H            # 512 gate rows
NH = 2                # batch halves (moving-dim chunks of 512)
NW = BL // NH         # 512

_f32 = mybir.dt.float32
_f32r = mybir.dt.float32r
_bf16 = mybir.dt.bfloat16

_CACHE: dict = {}


def _build():
    nc = bacc.Bacc("TRN2", target_bir_lowering=False, debug=False)
    AF = mybir.ActivationFunctionType

    zt_d = nc.dram_tensor("zt", [P, I, BL], _f32r, kind="ExternalInput")
    # packed inputs, one per DGE queue so the four prologue transfers run
    # in parallel: wz: [wz0 | wzf] f32r; wh0h: [wh0 | h0] bf16;
    # wrest: [whf | wo | c0] bf16. z_0 comes straight from zt.
    # weight layouts: columns are gate rows permuted to [f, i, g, o]
    wz_d = nc.dram_tensor("wzt", [I, 2 * G4], _f32r, kind="ExternalInput")
    wh0h_d = nc.dram_tensor("wh0ht", [H, G4 + BL], _bf16, kind="ExternalInput")
    wrest_d = nc.dram_tensor("wrestt", [H, G4 + H + BL], _bf16,
                             kind="ExternalInput")
    out_d = nc.dram_tensor("out", [P, BL], _f32, kind="ExternalOutput")

    with ExitStack() as ctx:
        tc = ctx.enter_context(tile.TileContext(nc))
        const = ctx.enter_context(tc.tile_pool(name="const", bufs=1))
        zp = ctx.enter_context(tc.tile_pool(name="z", bufs=4))
        hp = ctx.enter_context(tc.tile_pool(name="h", bufs=2))
        cp = ctx.enter_context(tc.tile_pool(name="c", bufs=2))
        gp = ctx.enter_context(tc.tile_pool(name="g", bufs=3))
        tp = ctx.enter_context(tc.tile_pool(name="t", bufs=3))
        op = ctx.enter_context(tc.tile_pool(name="osb", bufs=3))
        # PSUM budget (8 banks): FI 2x2 + G 2x1 + O 1x1 + po32 1 = 8
        psfi = ctx.enter_context(tc.tile_pool(name="psfi", bufs=2, space="PSUM"))
        psg = ctx.enter_context(tc.tile_pool(name="psg", bufs=2, space="PSUM"))
        pso = ctx.enter_context(tc.tile_pool(name="pso", bufs=1, space="PSUM"))
        pspo = ctx.enter_context(tc.tile_pool(name="pspo", bufs=1, space="PSUM"))

        # PE warmup: the tensor engine's clock ramps with sustained use
        # (p-state model: 0.65 -> 1.2 -> 2.4 GHz after 3us busy). Dummy
        # matmuls during the input-DMA window mean the real step-0 matmuls
        # start at full clock instead of 1.54 ns/col.
        wrm = tp.tile([H, H], _bf16, tag="wrm")
        nc.vector.memset(wrm[:], 0.0)
        wps = pspo.tile([H, H], _f32, tag="po32", name="warmup_ps")
        for _w in range(44):
            nc.tensor.matmul(wps[:], wrm[:], wrm[:], start=True, stop=True)

        # step-0-critical tensors first so the pipeline fills ASAP; four
        # packed transfers on four different DGE queues run in parallel.
        wzt = const.tile([I, 2 * G4], _f32r, tag="wzt")
        nc.sync.dma_start(wzt[:], wz_d[:])
        wz0 = wzt[:, 0:G4]
        wzf = wzt[:, G4 : 2 * G4]
        zt0 = zp.tile([I, BL], _f32r, tag="z", name="z0")
        nc.scalar.dma_start(zt0[:], zt_d[0, :, :])
        wh0ht = const.tile([H, G4 + BL], _bf16, tag="wh0ht")
        nc.gpsimd.dma_start(wh0ht[:], wh0h_d[:])
        wh0 = wh0ht[:, 0:G4]
        h_prev = wh0ht[:, G4 : G4 + BL]
        wrestt = const.tile([H, G4 + H + BL], _bf16, tag="wrestt")
        nc.sync.dma_start(wrestt[:], wrest_d[:])
        whf = wrestt[:, 0:G4]
        wo = wrestt[:, G4 : G4 + H]
        c_prev = wrestt[:, G4 + H : G4 + H + BL]

        def z_mms(t, zt, ps):
            """z-side (and bias) matmul contributions for step t; emitted
            during step t-1, they run while the PE waits for h_t. For t=0
            the matching h-side matmuls are interleaved per gate block so
            the first sigmoid evict isn't stuck behind all 16 z-matmuls on
            the cold (p-state-throttled) PE."""
            wz = wz0 if t == 0 else wzf
            nq = 2 if t == 0 else 1
            qw = NW // nq
            for half in range(NH):
                psFI = psfi.tile([H, 2 * NW], _f32, tag="fi",
                                 name=f"psFI{t}_{half}")
                psG = psg.tile([H, NW], _f32, tag="g", name=f"psG{t}_{half}")
                psO = pso.tile([H, NW], _f32, tag="o", name=f"psO{t}_{half}")
                ps[(t, half)] = (psFI, psG, psO)
                for j in range(2):      # f, i blocks
                    for q in range(nq):
                        js = slice(j * NW + q * qw, j * NW + (q + 1) * qw)
                        qs = slice(half * NW + q * qw, half * NW + (q + 1) * qw)
                        nc.tensor.matmul(psFI[:, js], wz[:, j * H : (j + 1) * H],
                                         zt[:, qs], start=(q == 0), stop=False)
                if t == 0:
                    h_mms_fi(t, half, psFI)
                for q in range(nq):
                    qs = slice(half * NW + q * qw, half * NW + (q + 1) * qw)
                    qj = slice(q * qw, (q + 1) * qw)
                    nc.tensor.matmul(psG[:, qj], wz[:, 2 * H : 3 * H], zt[:, qs],
                                     start=(q == 0), stop=False)
                    nc.tensor.matmul(psO[:, qj], wz[:, 3 * H : 4 * H], zt[:, qs],
                                     start=(q == 0), stop=False)
                if t == 0:
                    h_mms_go(t, half, psG, psO)

        def h_mms_fi(t, half, psFI):
            """h-side f,i matmuls in 256-col quarters (h lands in quarter
            chunks from the split h-mul, so the first mms start early and
            the sFI evict unblocks sooner)."""
            wh = wh0 if t == 0 else whf
            hw_ = NW // 2
            for q in range(2):
                for j in range(2):
                    js = slice(j * NW + q * hw_, j * NW + (q + 1) * hw_)
                    qs = slice(half * NW + q * hw_, half * NW + (q + 1) * hw_)
                    nc.tensor.matmul(psFI[:, js], wh[:, j * H : (j + 1) * H],
                                     h_prev[:, qs], start=False, stop=True)

        def h_mms_go(t, half, psG, psO):
            wh = wh0 if t == 0 else whf
            cs = slice(half * NW, (half + 1) * NW)
            nc.tensor.matmul(psG[:], wh[:, 2 * H : 3 * H], h_prev[:, cs],
                             start=False, stop=True)
            nc.tensor.matmul(psO[:], wh[:, 3 * H : 4 * H], h_prev[:, cs],
                             start=False, stop=True)

        ps: dict = {}
        z_mms(0, zt0, ps)

        po32: dict = {}

        _PO_GROUPS = {}
        for _g0, _glen in ((0, 32), (32, 31), (63, 1)):
            for _t in range(_g0, _g0 + _glen):
                _PO_GROUPS[_t] = (_g0, _glen)

        def emit_po(tp_, h_tile):
            g0, glen = _PO_GROUPS[tp_]
            j = tp_ - g0
            if j == 0:
                po32[0] = pspo.tile([64, NW], _f32, tag="po32",
                                    name=f"po32_{tp_}")
            for half in range(NH):
                cs = slice(half * NW, (half + 1) * NW)
                blk = 63 - (half * 32 + j)
                nc.tensor.matmul(po32[0][:], wo[:, blk : blk + 64],
                                 h_tile[:, cs],
                                 start=(j == 0 and half == 0),
                                 stop=(j == glen - 1 and half == NH - 1))
            if j == glen - 1:
                orow32 = op.tile([64, NW], _f32, tag="orow", name=f"orow{tp_}")
                nc.vector.tensor_copy(orow32[:], po32[0][:])
                if glen == 1:
                    nc.sync.dma_start(out_d[g0 : g0 + 1, :],
                                      orow32[0:64:32, :])
                else:
                    for half in range(NH):
                        cs = slice(half * NW, (half + 1) * NW)
                        nc.sync.dma_start(out_d[g0 : g0 + glen, cs],
                                          orow32[32 * half : 32 * half + glen, :])

        prev = None  # (t, h_tile) pending out-projection
        for t in range(P):
            h_new = hp.tile([H, BL], _bf16, tag="h", name=f"h{t}")
            c_new = cp.tile([H, BL], _bf16, tag="c", name=f"c{t}")
            gFI = [None, None]
            gG = [None, None]
            gO = [None, None]
            ct = [None, None]

            def evict_fi(half):
                psFI, _, _ = ps[(t, half)]
                gFI[half] = gp.tile([H, 2 * NW], _bf16, tag="gFI",
                                    name=f"gFI{t}_{half}")
                nc.scalar.activation(gFI[half][:], psFI[:], AF.Sigmoid)

            def evict_g(half):
                _, psG, _ = ps[(t, half)]
                gG[half] = gp.tile([H, NW], _bf16, tag="gG",
                                   name=f"gG{t}_{half}")
                nc.scalar.activation(gG[half][:], psG[:], AF.Tanh)

            def evict_o(half):
                _, _, psO = ps[(t, half)]
                gO[half] = gp.tile([H, NW], _bf16, tag="gO",
                                   name=f"gO{t}_{half}")
                nc.scalar.activation(gO[half][:], psO[:], AF.Sigmoid)

            def cell(half):
                """c = f*c_prev + i*g, all on DVE (t1 fp32, t2 bf16 2x)."""
                cs = slice(half * NW, (half + 1) * NW)
                f_s = gFI[half][:, 0:NW]
                i_s = gFI[half][:, NW : 2 * NW]
                t1 = tp.tile([H, NW], _bf16, tag="t1", name=f"t1_{t}_{half}")
                nc.vector.tensor_mul(t1[:], f_s, c_prev[:, cs])
                t2 = tp.tile([H, NW], _bf16, tag="t2", name=f"t2_{t}_{half}")
                nc.vector.tensor_mul(t2[:], i_s, gG[half][:])
                nc.vector.tensor_add(c_new[:, cs], t1[:], t2[:])

            def tanh_c(half):
                cs = slice(half * NW, (half + 1) * NW)
                ct[half] = tp.tile([H, NW], _bf16, tag="ct",
                                   name=f"ct{t}_{half}")
                nc.scalar.activation(ct[half][:], c_new[:, cs], AF.Tanh)

            def h_mul(half):
                """h = o * tanh(c) in two 256-col chunks so the next step's
                f,i matmuls can start on the first chunk early."""
                hw_ = NW // 2
                for q in range(2):
                    qs = slice(half * NW + q * hw_, half * NW + (q + 1) * hw_)
                    qq = slice(q * hw_, (q + 1) * hw_)
                    nc.vector.tensor_mul(h_new[:, qs], gO[half][:, qq],
                                         ct[half][:, qq])

            # ACT priority order: sFI0 G0 sO0 sFI1 G1 ct0 sO1 ct1
            if t > 0:
                psFI0, psG0, psO0 = ps[(t, 0)]
                h_mms_fi(t, 0, psFI0)
                h_mms_go(t, 0, psG0, psO0)
            evict_fi(0)
            evict_g(0)
            evict_o(0)
            cell(0)
            if t > 0:
                psFI1, psG1, psO1 = ps[(t, 1)]
                h_mms_fi(t, 1, psFI1)
                h_mms_go(t, 1, psG1, psO1)
            evict_fi(1)
            evict_g(1)
            tanh_c(0)
            cell(1)
            h_mul(0)
            evict_o(1)
            tanh_c(1)
            h_mul(1)
            ps.pop((t, 0))
            ps.pop((t, 1))
            if t + 1 < P:
                zt = zp.tile([I, BL], _f32r, tag="z", name=f"z{t + 1}")
                nc.sync.dma_start(zt[:], zt_d[t + 1, :, :])
                z_mms(t + 1, zt, ps)
            if prev is not None:
                emit_po(prev[0], prev[1])
            prev = (t, h_new)
            h_prev, c_prev = h_new, c_new
        emit_po(prev[0], prev[1])

    nc.compile()
    return nc


def _get_nc():
    if "nc" not in _CACHE:
        _CACHE["nc"] = _build()
    return _CACHE["nc"]


# gate-row permutation: PyTorch order [i,f,g,o] -> kernel order [f,i,g,o]
_PERM = np.concatenate(
    [np.arange(H, 2 * H), np.arange(0, H), np.arange(2 * H, 3 * H),
     np.arange(3 * H, 4 * H)]
)


def _prep_in_maps(x, z, h0, c0, W_ih, W_hh, b_ih, b_hh, W_out, b_out):
    f = np.float32
    Wihp = W_ih[_PERM]                                   # (512, 64)
    Whhp = W_hh[_PERM]                                   # (512, 128)
    Whfp = Whhp + Wihp[:, 0:1] @ W_out                   # fold out-projection
    b0 = (b_ih + b_hh)[_PERM].astype(f)
    bf = (b0 + Wihp[:, 0] * b_out[0]).astype(f)

    wz0t = np.concatenate([Wihp.T, b0[None, :]], axis=0).astype(f)   # (65, 512)
    wzft = np.concatenate([Wihp.T, bf[None, :]], axis=0).astype(f)   # (65, 512)
    whh0t = np.ascontiguousarray(Whhp.T, dtype=f)                    # (128, 512)
    whhft = np.ascontiguousarray(Whfp.T, dtype=f)                    # (128, 512)
    woutt = np.zeros((H, H), dtype=f)
    woutt[:, 63] = W_out[0]

    import ml_dtypes
    in_maps = []
    for m in range(NCORES):
        sl = slice(m * BL, (m + 1) * BL)
        z_aug = np.empty((P, I, BL), dtype=f)
        z_aug[:, 0, :] = 0.0
        z_aug[0, 0, :] = x[sl, -1, 0]
        z_aug[:, 1:-1, :] = np.transpose(z[sl, T - P :, :], (1, 2, 0))
        z_aug[:, -1, :] = 1.0
        wh0h = np.concatenate([whh0t, h0[0, sl, :].T], axis=1)   # (128, 1536)
        wrest = np.concatenate(
            [whhft, woutt, c0[0, sl, :].T], axis=1)              # (128, 1664)
        in_maps.append(
            {
                "zt": np.ascontiguousarray(z_aug),
                "wzt": np.ascontiguousarray(
                    np.concatenate([wz0t, wzft], axis=1)),       # (65, 1024)
                "wh0ht": np.ascontiguousarray(wh0h).astype(ml_dtypes.bfloat16),
                "wrestt": np.ascontiguousarray(wrest).astype(ml_dtypes.bfloat16),
            }
        )
    return in_maps


def run_on_cores(inputs: dict, **spmd_kwargs):
    """Build + run; returns (full_output, BassKernelResults)."""
    inputs = {k: np.asarray(v, dtype=np.float32) for k, v in inputs.items()}
    nc = _get_nc()
    in_maps = _prep_in_maps(**inputs)
    res = run_bass_kernel_spmd(nc, in_maps, core_ids=list(range(NCORES)), **spmd_kwargs)
    outs = np.concatenate(
        [r["out"].T for r in res.results], axis=0
    )  # (8192, 64)
    outs = outs + np.float32(inputs["b_out"][0])
    return outs[:, :, None].astype(np.float32), res


def kernel(**inputs) -> np.ndarray:
    out, _ = run_on_cores(inputs)
    return out


# revision 13
# speedup vs baseline: 1.3406x; 1.0216x over previous
"""Trainium2 Bass kernel for nn_DecoderRNN (autoregressive LSTM decoder).

Strategy:
  - Pure data parallelism: batch 8192 -> 1024 per core across 8 NeuronCores.
  - Feature-major layout on chip: h^T, c^T are [H=128 partitions, B_local].
    gates^T = W^T blocks (stationary) @ activations (moving), so the
    elementwise LSTM cell update produces h^T directly in the layout the
    next step's matmul needs -- no per-step transposes.
  - Output projection folded into the recurrent weights (W~_hh = W_hh +
    W_ih[:,0:1] @ W_out); biases ride in the matmul via a ones-row in the
    z tile. Step 0 uses unfolded weights with x supplied via the z tile.
  - ACT (scalar engine) is the bottleneck: 5 transcendental passes over
    [128,1024] per step = 4.27us/step of column time at 1.2GHz, plus
    ~185ns fixed cost per activation instruction. The schedule targets
    exactly 8 ACT ops/step (sFI, G, sO, ct per half) in the stream order
      sFI0 G0 sO0 sFI1 G1 ct0 sO1 ct1
    which keeps every op's inputs ready >= its slot start and closes the
    per-half recurrence cycle (evict -> cell -> tanh(c) -> h -> h-side
    matmul -> next evict) in exactly the 6.0us/step ACT busy time.
  - Gate order [f, i, g, o]: f,i share a 2-bank PSUM tile (one sigmoid
    evict); g and o are 1-bank tiles. tanh(c) is one [128,512] op per
    half (not quartered: fewer ACT ops wins over finer pipelining).
  - Gate evictions write bf16: i*g runs in the DVE 2x packed mode, and
    h = o*tanh(c) is all-bf16 (h feeds the matmuls as bf16 moving data,
    same 1 cycle/row as f32r). c and f*c stay fp32 for accuracy.
  - All three cell ops (f*c, i*g, add) run on DVE -- GPSIMD's 0.42x
    multiply efficiency makes it too slow for the critical path; it only
    gets the out-row PSUM->SBUF copies.
  - z-side matmuls for step t+1 are pre-issued during step t; h-side
    matmuls are emitted f,i,g,o so the sFI evict unblocks after two mms.
  - Out rows: step t / half b lands on PSUM partition 32*b + t%32 via
    shifted W_out column blocks, so 64 rows accumulate in one PSUM bank
    and evict once per 32 steps.
  - PSUM budget (8 banks): FI 2bufs x 2 + G 2bufs x 1 + O 1buf x 1 +
    po32 1 = 8.
"""

import os
import sys

for _p in ("/opt/trn_rl_repo", "/root/.axon_site/_ro/trn_rl_repo"):
    if os.path.isdir(_p) and _p not in sys.path:
        sys.path.insert(0, _p)

from contextlib import ExitStack

import numpy as np

import concourse.bass as bass  # noqa: F401  (registers types)
import concourse.mybir as mybir
import concourse.tile as tile
from concourse import bacc
from concourse.bass_utils import run_bass_kernel_spmd

NCORES = 8
B, T, F, H, P = 8192, 128, 63, 128, 64
BL = B // NCORES      # 1024 rows per core
I = 2 + F             # 64 LSTM input features + 1 ones-row for bias
G4 = 4 * H            # 512 gate rows
NH = 2                # batch halves (moving-dim chunks of 512)
NW = BL // NH         # 512

_f32 = mybir.dt.float32
_f32r = mybir.dt.float32r
_bf16 = mybir.dt.bfloat16

_CACHE: dict = {}


def _build():
    nc = bacc.Bacc("TRN2", target_bir_lowering=False, debug=False)
    AF = mybir.ActivationFunctionType

    zt_d = nc.dram_tensor("zt", [P, I, BL], _f32r, kind="ExternalInput")
    # packed inputs, one per DGE queue so the four prologue transfers run
    # in parallel: wz: [wz0 | wzf] f32r; wh0h: [wh0 | h0] bf16;
    # wrest: [whf | wo | c0] bf16. z_0 comes straight from zt.
    # weight layouts: columns are gate rows permuted to [f, i, g, o]
    wz_d = nc.dram_tensor("wzt", [I, 2 * G4], _f32r, kind="ExternalInput")
    wh0h_d = nc.dram_tensor("wh0ht", [H, G4 + BL], _bf16, kind="ExternalInput")
    wrest_d = nc.dram_tensor("wrestt", [H, G4 + H + BL], _bf16,
                             kind="ExternalInput")
    out_d = nc.dram_tensor("out", [P, BL], _f32, kind="ExternalOutput")

    with ExitStack() as ctx:
        tc = ctx.enter_context(tile.TileContext(nc))
        const = ctx.enter_context(tc.tile_pool(name="const", bufs=1))
        zp = ctx.enter_context(tc.tile_pool(name="z", bufs=4))
        hp = ctx.enter_context(tc.tile_pool(name="h", bufs=2))
        cp = ctx.enter_context(tc.tile_pool(name="c", bufs=2))
        gp = ctx.enter_context(tc.tile_pool(name="g", bufs=3))
        tp = ctx.enter_context(tc.tile_pool(name="t", bufs=3))
        op = ctx.enter_context(tc.tile_pool(name="osb", bufs=3))
        # PSUM budget (8 banks): FI 2x2 + G 2x1 + O 1x1 + po32 1 = 8
        psfi = ctx.enter_context(tc.tile_pool(name="psfi", bufs=2, space="PSUM"))
        psg = ctx.enter_context(tc.tile_pool(name="psg", bufs=1, space="PSUM"))
        pso = ctx.enter_context(tc.tile_pool(name="pso", bufs=2, space="PSUM"))
        pspo = ctx.enter_context(tc.tile_pool(name="pspo", bufs=1, space="PSUM"))

        # PE warmup: the tensor engine's clock ramps with sustained use
        # (p-state model: 0.65 -> 1.2 -> 2.4 GHz after 3us busy). Dummy
        # matmuls during the input-DMA window mean the real step-0 matmuls
        # start at full clock instead of 1.54 ns/col.
        wrm = tp.tile([H, H], _bf16, tag="wrm")
        nc.vector.memset(wrm[:], 0.0)
        wps = pspo.tile([H, H], _f32, tag="po32", name="warmup_ps")
        for _w in range(44):
            nc.tensor.matmul(wps[:], wrm[:], wrm[:], start=True, stop=True)

        # step-0-critical tensors first so the pipeline fills ASAP; four
        # packed transfers on four different DGE queues run in parallel.
        wzt = const.tile([I, 2 * G4], _f32r, tag="wzt")
        nc.sync.dma_start(wzt[:], wz_d[:])
        wz0 = wzt[:, 0:G4]
        wzf = wzt[:, G4 : 2 * G4]
        zt0 = zp.tile([I, BL], _f32r, tag="z", name="z0")
        nc.scalar.dma_start(zt0[:], zt_d[0, :, :])
        wh0ht = const.tile([H, G4 + BL], _bf16, tag="wh0ht")
        nc.gpsimd.dma_start(wh0ht[:], wh0h_d[:])
        wh0 = wh0ht[:, 0:G4]
        h_prev = wh0ht[:, G4 : G4 + BL]
        wrestt = const.tile([H, G4 + H + BL], _bf16, tag="wrestt")
        nc.sync.dma_start(wrestt[:], wrest_d[:])
        whf = wrestt[:, 0:G4]
        wo = wrestt[:, G4 : G4 + H]
        c_prev = wrestt[:, G4 + H : G4 + H + BL]

        def z_mms(t, zt, ps):
            """z-side (and bias) matmul contributions for step t; emitted
            during step t-1, they run while the PE waits for h_t. For t=0
            the matching h-side matmuls are interleaved per gate block so
            the first sigmoid evict isn't stuck behind all 16 z-matmuls on
            the cold (p-state-throttled) PE."""
            wz = wz0 if t == 0 else wzf
            nq = 2 if t == 0 else 1
            qw = NW // nq
            for half in range(NH):
                psFI = psfi.tile([H, 2 * NW], _f32, tag="fi",
                                 name=f"psFI{t}_{half}")
                psG = psg.tile([H, NW], _f32, tag="g", name=f"psG{t}_{half}")
                psO = pso.tile([H, NW], _f32, tag="o", name=f"psO{t}_{half}")
                ps[(t, half)] = (psFI, psG, psO)
                for j in range(2):      # f, i blocks
                    for q in range(nq):
                        js = slice(j * NW + q * qw, j * NW + (q + 1) * qw)
                        qs = slice(half * NW + q * qw, half * NW + (q + 1) * qw)
                        nc.tensor.matmul(psFI[:, js], wz[:, j * H : (j + 1) * H],
                                         zt[:, qs], start=(q == 0), stop=False)
                if t == 0:
                    h_mms_fi(t, half, psFI)
                for q in range(nq):
                    qs = slice(half * NW + q * qw, half * NW + (q + 1) * qw)
                    qj = slice(q * qw, (q + 1) * qw)
                    nc.tensor.matmul(psG[:, qj], wz[:, 2 * H : 3 * H], zt[:, qs],
                                     start=(q == 0), stop=False)
                    nc.tensor.matmul(psO[:, qj], wz[:, 3 * H : 4 * H], zt[:, qs],
                                     start=(q == 0), stop=False)
                if t == 0:
                    h_mms_go(t, half, psG, psO)

        def h_mms_fi(t, half, psFI):
            """h-side f,i matmuls in 256-col quarters (h lands in quarter
            chunks from the split h-mul, so the first mms start early and
            the sFI evict unblocks sooner)."""
            wh = wh0 if t == 0 else whf
            hw_ = NW // 2
            for q in range(2):
                for j in range(2):
                    js = slice(j * NW + q * hw_, j * NW + (q + 1) * hw_)
                    qs = slice(half * NW + q * hw_, half * NW + (q + 1) * hw_)
                    nc.tensor.matmul(psFI[:, js], wh[:, j * H : (j + 1) * H],
                                     h_prev[:, qs], start=False, stop=True)

        def h_mms_go(t, half, psG, psO):
            wh = wh0 if t == 0 else whf
            cs = slice(half * NW, (half + 1) * NW)
            nc.tensor.matmul(psG[:], wh[:, 2 * H : 3 * H], h_prev[:, cs],
                             start=False, stop=True)
            nc.tensor.matmul(psO[:], wh[:, 3 * H : 4 * H], h_prev[:, cs],
                             start=False, stop=True)

        ps: dict = {}
        z_mms(0, zt0, ps)

        po32: dict = {}

        _PO_GROUPS = {}
        for _g0, _glen in ((0, 32), (32, 31), (63, 1)):
            for _t in range(_g0, _g0 + _glen):
                _PO_GROUPS[_t] = (_g0, _glen)

        def emit_po(tp_, h_tile):
            g0, glen = _PO_GROUPS[tp_]
            j = tp_ - g0
            if j == 0:
                po32[0] = pspo.tile([64, NW], _f32, tag="po32",
                                    name=f"po32_{tp_}")
            for half in range(NH):
                cs = slice(half * NW, (half + 1) * NW)
                blk = 63 - (half * 32 + j)
                nc.tensor.matmul(po32[0][:], wo[:, blk : blk + 64],
                                 h_tile[:, cs],
                                 start=(j == 0 and half == 0),
                                 stop=(j == glen - 1 and half == NH - 1))
            if j == glen - 1:
                orow32 = op.tile([64, NW], _f32, tag="orow", name=f"orow{tp_}")
                nc.vector.tensor_copy(orow32[:], po32[0][:])
                if glen == 1:
                    nc.sync.dma_start(out_d[g0 : g0 + 1, :],
                                      orow32[0:64:32, :])
                else:
                    for half in range(NH):
                        cs = slice(half * NW, (half + 1) * NW)
                        nc.sync.dma_start(out_d[g0 : g0 + glen, cs],
                                          orow32[32 * half : 32 * half + glen, :])

        prev = None  # (t, h_tile) pending out-projection
        for t in range(P):
            h_new = hp.tile([H, BL], _bf16, tag="h", name=f"h{t}")
            c_new = cp.tile([H, BL], _bf16, tag="c", name=f"c{t}")
            gFI = [None, None]
            gG = [None, None]
            gO = [None, None]
            ct = [None, None]

            def evict_fi(half):
                psFI, _, _ = ps[(t, half)]
                gFI[half] = gp.tile([H, 2 * NW], _bf16, tag="gFI",
                                    name=f"gFI{t}_{half}")
                nc.scalar.activation(gFI[half][:], psFI[:], AF.Sigmoid)

            def evict_g(half):
                _, psG, _ = ps[(t, half)]
                gG[half] = gp.tile([H, NW], _bf16, tag="gG",
                                   name=f"gG{t}_{half}")
                nc.scalar.activation(gG[half][:], psG[:], AF.Tanh)

            def evict_o(half):
                _, _, psO = ps[(t, half)]
                gO[half] = gp.tile([H, NW], _bf16, tag="gO",
                                   name=f"gO{t}_{half}")
                nc.scalar.activation(gO[half][:], psO[:], AF.Sigmoid)

            def cell(half):
                """c = f*c_prev + i*g, all on DVE (t1 fp32, t2 bf16 2x)."""
                cs = slice(half * NW, (half + 1) * NW)
                f_s = gFI[half][:, 0:NW]
                i_s = gFI[half][:, NW : 2 * NW]
                t1 = tp.tile([H, NW], _bf16, tag="t1", name=f"t1_{t}_{half}")
                nc.vector.tensor_mul(t1[:], f_s, c_prev[:, cs])
                t2 = tp.tile([H, NW], _bf16, tag="t2", name=f"t2_{t}_{half}")
                nc.vector.tensor_mul(t2[:], i_s, gG[half][:])
                nc.vector.tensor_add(c_new[:, cs], t1[:], t2[:])

            def tanh_c(half):
                cs = slice(half * NW, (half + 1) * NW)
                ct[half] = tp.tile([H, NW], _bf16, tag="ct",
                                   name=f"ct{t}_{half}")
                nc.scalar.activation(ct[half][:], c_new[:, cs], AF.Tanh)

            def h_mul(half):
                """h = o * tanh(c) in two 256-col chunks so the next step's
                f,i matmuls can start on the first chunk early."""
                hw_ = NW // 2
                for q in range(2):
                    qs = slice(half * NW + q * hw_, half * NW + (q + 1) * hw_)
                    qq = slice(q * hw_, (q + 1) * hw_)
                    nc.vector.tensor_mul(h_new[:, qs], gO[half][:, qq],
                                         ct[half][:, qq])

            # ACT priority order: sFI0 G0 sFI1 G1 sO0 ct0 sO1 ct1.
            # G1 early pulls half-1's i*g / c-add forward on the DVE, so the
            # h0 mul later finds the DVE free and the return path (ct0 ->
            # h0 -> f,i matmuls -> sFI0 of step t+1) fits inside the sO1+ct1
            # ACT slots.
            if t > 0:
                psFI0, psG0, psO0 = ps[(t, 0)]
                h_mms_fi(t, 0, psFI0)
                h_mms_go(t, 0, psG0, psO0)
            evict_fi(0)
            evict_g(0)
            cell(0)
            if t > 0:
                psFI1, psG1, psO1 = ps[(t, 1)]
                h_mms_fi(t, 1, psFI1)
                h_mms_go(t, 1, psG1, psO1)
            evict_fi(1)
            evict_g(1)
            evict_o(0)
            tanh_c(0)
            cell(1)
            h_mul(0)
            evict_o(1)
            tanh_c(1)
            h_mul(1)
            ps.pop((t, 0))
            ps.pop((t, 1))
            if t + 1 < P:
                zt = zp.tile([I, BL], _f32r, tag="z", name=f"z{t + 1}")
                nc.sync.dma_start(zt[:], zt_d[t + 1, :, :])
                z_mms(t + 1, zt, ps)
            if prev is not None:
                emit_po(prev[0], prev[1])
            prev = (t, h_new)
            h_prev, c_prev = h_new, c_new
        emit_po(prev[0], prev[1])

    nc.compile()
    return nc


def _get_nc():
    if "nc" not in _CACHE:
        _CACHE["nc"] = _build()
    return _CACHE["nc"]


# gate-row permutation: PyTorch order [i,f,g,o] -> kernel order [f,i,g,o]
_PERM = np.concatenate(
    [np.arange(H, 2 * H), np.arange(0, H), np.arange(2 * H, 3 * H),
     np.arange(3 * H, 4 * H)]
)


def _prep_in_maps(x, z, h0, c0, W_ih, W_hh, b_ih, b_hh, W_out, b_out):
    f = np.float32
    Wihp = W_ih[_PERM]                                   # (512, 64)
    Whhp = W_hh[_PERM]                                   # (512, 128)
    Whfp = Whhp + Wihp[:, 0:1] @ W_out                   # fold out-projection
    b0 = (b_ih + b_hh)[_PERM].astype(f)
    bf = (b0 + Wihp[:, 0] * b_out[0]).astype(f)

    wz0t = np.concatenate([Wihp.T, b0[None, :]], axis=0).astype(f)   # (65, 512)
    wzft = np.concatenate([Wihp.T, bf[None, :]], axis=0).astype(f)   # (65, 512)
    whh0t = np.ascontiguousarray(Whhp.T, dtype=f)                    # (128, 512)
    whhft = np.ascontiguousarray(Whfp.T, dtype=f)                    # (128, 512)
    woutt = np.zeros((H, H), dtype=f)
    woutt[:, 63] = W_out[0]

    import ml_dtypes
    in_maps = []
    for m in range(NCORES):
        sl = slice(m * BL, (m + 1) * BL)
        z_aug = np.empty((P, I, BL), dtype=f)
        z_aug[:, 0, :] = 0.0
        z_aug[0, 0, :] = x[sl, -1, 0]
        z_aug[:, 1:-1, :] = np.transpose(z[sl, T - P :, :], (1, 2, 0))
        z_aug[:, -1, :] = 1.0
        wh0h = np.concatenate([whh0t, h0[0, sl, :].T], axis=1)   # (128, 1536)
        wrest = np.concatenate(
            [whhft, woutt, c0[0, sl, :].T], axis=1)              # (128, 1664)
        in_maps.append(
            {
                "zt": np.ascontiguousarray(z_aug),
                "wzt": np.ascontiguousarray(
                    np.concatenate([wz0t, wzft], axis=1)),       # (65, 1024)
                "wh0ht": np.ascontiguousarray(wh0h).astype(ml_dtypes.bfloat16),
                "wrestt": np.ascontiguousarray(wrest).astype(ml_dtypes.bfloat16),
            }
        )
    return in_maps


def run_on_cores(inputs: dict, **spmd_kwargs):
    """Build + run; returns (full_output, BassKernelResults)."""
    inputs = {k: np.asarray(v, dtype=np.float32) for k, v in inputs.items()}
    nc = _get_nc()
    in_maps = _prep_in_maps(**inputs)
    res = run_bass_kernel_spmd(nc, in_maps, core_ids=list(range(NCORES)), **spmd_kwargs)
    outs = np.concatenate(
        [r["out"].T for r in res.results], axis=0
    )  # (8192, 64)
    outs = outs + np.float32(inputs["b_out"][0])
    return outs[:, :, None].astype(np.float32), res


def kernel(**inputs) -> np.ndarray:
    out, _ = run_on_cores(inputs)
    return out
